# revision 61
# baseline (speedup 1.0000x reference)
"""Trainium2 Bass kernel for the attention module (b=4, c=256, l=2048, h=8, d=64).

Sharding: 8 cores = 4 batches x 2 query-halves (no collectives). Each core
receives its batch's x with columns permuted so its own query half comes
first; it computes k/v for all 2048 key positions and the attention output
for its 1024 queries, then the output projection + final rms-norm.

Device algorithm per core (all matmul data bf16, psum fp32):
  ss     = ones128.T @ bf16(x*x)            (PE bcast-reduce: [128,*] rows
                                             all equal to column sum-sq)
  s      = exp(-0.5*ln(ss/256+eps))         (ACT Ln + ACT Exp: rsqrt using
                                             the single natural_log_exp
                                             table -- no mid-kernel ACT
                                             table swaps, ever)
  xn     = bf16(x * s)                      (DVE, s pre-broadcast)
  q,k    = bf16(wqkvT-slices.T @ xn)        (g1 folded on host)
  vT     = bf16(xn_slice.T @ wv)
  per head:  simT[j,i] = k_h.T @ q_h        (psum fp32, logits*8)
             pt = exp(simT/8) as bf16       (split across ACT true exp, DVE
                                             Schraudolph bitcast-exp, and a
                                             DMA->SBUF + Pool Schraudolph
                                             path so no engine gates PE)
             ytps[i, d] += pt_chunk.T @ vT_h
  y_h    = ytps[:, 0:64] * (1/den) broadcast (DVE, stride-0 bcast; den
                                             rides yt as a fused ones col)
  y4     = PE transpose(yts, identity)      (bf16, per head-pair)
  out    = woT.T @ y4 + b_out
  result = out * g2 * exp(-0.5*ln(ss2/256+eps))   (bcast-reduce again)

Scheduling: 512-col-chunked input norm so the first q/k projections start
~4us in; one rotating 3x[128,1024] psum arena for projections, sim chunks,
pair transposes and the out-proj; head h's sim+exp stream overlaps head
h-1's yT accumulation; 256-col-pipelined output tail (out-proj pair-3,
bias, sum-sq, ln/exp, scale, store per block) to cut the serial tail.
Late-consumed tensors (idn/wo/bo/g2) are DMA'd on the gpsimd SWDGE queue.
"""
import sys

import numpy as np

if "/opt/trn_rl_repo" not in sys.path:
    sys.path.insert(0, "/opt/trn_rl_repo")

import ml_dtypes  # noqa: E402

import concourse.bass as bass  # noqa: E402
import concourse.tile as tile  # noqa: E402
from concourse import bacc, mybir  # noqa: E402
from concourse.bass_utils import run_bass_kernel_spmd  # noqa: E402

F32 = mybir.dt.float32
F32R = mybir.dt.float32r
BF16 = mybir.dt.bfloat16
U16 = mybir.dt.uint16
AF = mybir.ActivationFunctionType
MUL = mybir.AluOpType.mult
ADD = mybir.AluOpType.add

B, C, L = 4, 256, 2048
H, D = 8, 64
HID = H * D
LQ = L // 2      # queries per core
NCT = C // 128   # 2 c-tiles
NJ = L // 128    # 16 j-tiles
NI = LQ // 128   # 8 i-chunks
EPS_B = 1e-26

LOG2E = 1.4426950408889634
# Schraudolph bf16: bits = round(logit*128*log2e + 127*128 - 5.6); logit = sim/8
SCH_A = 128.0 * LOG2E / 8.0
SCH_B = 127.0 * 128.0 - 5.6

# exp engine split per (h, jt): 'A' ACT true exp, 'V' DVE Schraudolph.
# (PSUM cannot be DMA'd and Pool cannot read PSUM, so these two engines
# carry all 128 exp tiles; the ratio balances their total load just under
# the PE roofline.) Alternation keeps the 3-deep sim psum ring draining
# on both engines at once.
_ROW8A = ['A', 'V', 'A', 'V', 'A', 'V', 'A', 'V',
          'A', 'V', 'A', 'V', 'A', 'V', 'A', 'V']
_ROW9A = ['A', 'V', 'A', 'V', 'A', 'A', 'V', 'A',
          'V', 'A', 'A', 'V', 'A', 'V', 'A', 'A']
# h0's DVE is busy with the s-folded q/k conversions until ~jt6, so its
# early exps go to ACT.
_ROWH0 = ['A', 'A', 'A', 'A', 'A', 'A', 'V', 'A',
          'V', 'A', 'V', 'A', 'V', 'A', 'V', 'A']
EXP_ENG = [list(_ROWH0)] + [list(_ROW9A) if h % 2 else list(_ROW8A)
                            for h in range(1, H)]


def _body(tc, x, wq, wo, bo, g2, idn, out):
    nc = tc.nc
    from contextlib import ExitStack
    with ExitStack() as ctx:
        ctx.enter_context(nc.allow_low_precision(
            reason="bf16 data path by design"))
        const = ctx.enter_context(tc.tile_pool(name="const", bufs=1))
        bigx = ctx.enter_context(tc.tile_pool(name="bigx", bufs=1))
        sqp = ctx.enter_context(tc.tile_pool(name="sq", bufs=2))
        slnp = ctx.enter_context(tc.tile_pool(name="sln", bufs=2))
        sbnp = ctx.enter_context(tc.tile_pool(name="sbn", bufs=1))
        sbcp = ctx.enter_context(tc.tile_pool(name="sbc", bufs=4))
        qp = ctx.enter_context(tc.tile_pool(name="q", bufs=4))
        kp = ctx.enter_context(tc.tile_pool(name="k", bufs=4))
        vtp = ctx.enter_context(tc.tile_pool(name="vt", bufs=16))
        ptp = ctx.enter_context(tc.tile_pool(name="pt", bufs=2))
        invp = ctx.enter_context(tc.tile_pool(name="inv", bufs=2))
        ytsp = ctx.enter_context(tc.tile_pool(name="yts", bufs=2))
        y4p = ctx.enter_context(tc.tile_pool(name="y4", bufs=4))
        outp = ctx.enter_context(tc.tile_pool(name="outp", bufs=1))
        finp = ctx.enter_context(tc.tile_pool(name="fin", bufs=2))

        # ---------------- constants ----------------
        # Preload the one ACT table that covers every function this kernel
        # uses (Ln, Exp, Copy, Identity). Bacc's auto-inserter is greedy
        # per-function and would thrash 13 loads between the single-function
        # tables; with this explicit load it sees every activation covered.
        from concourse.hw_specs import get_activation_tables
        tabs = list(get_activation_tables(nc.m.arch))
        joint_id = tabs.index("natural_log_exp_and_others")
        nc.scalar.add_instruction(mybir.InstLoadActFuncSet(
            name=nc.get_next_instruction_name(),
            act_func_set_id=joint_id, engine=mybir.EngineType.Activation,
            ins=[], outs=[]))

        dmyr = const.tile([128, 512], BF16, tag="dmyr")
        nc.vector.memset(dmyr[:], 0.0)
        ones128 = const.tile([128, 128], BF16, tag="ones128")
        nc.vector.memset(ones128[:], 1.0)
        stage = const.tile([128, 8], F32, tag="stage")
        nc.gpsimd.memset(stage[:], 1.0)
        eps_t = const.tile([128, 1], F32, tag="eps")
        nc.gpsimd.memset(eps_t[:], EPS_B)
        zero_t = const.tile([128, 1], F32, tag="zero")
        nc.gpsimd.memset(zero_t[:], 0.0)

        # ---------------- input DMAs ----------------
        # x as four 512-col chunk DMAs, each moving BOTH c-tiles in one
        # transfer (256 dram rows -> [128, 2, 512] sbuf) so only 4 HWDGE
        # setups serialize. wq rides the DVE queue so the ACT sequencer
        # stays free for the norm-chain Ln/Exp.
        x_sb = bigx.tile([128, NCT, L], BF16, tag="x")
        wq_sb = []
        for ct in range(NCT):
            wq_sb.append(const.tile([128, 3 * HID], BF16, tag=f"wq{ct}",
                                    name=f"wq{ct}"))
        nc.sync.dma_start(
            x_sb[:, :, 0:1024],
            x[:, 0:1024].rearrange("(a p) c -> p a c", a=NCT))
        for ct in range(NCT):
            nc.scalar.dma_start(wq_sb[ct][:, 0:HID],
                                wq[ct * 128:(ct + 1) * 128, 0:HID])
        nc.sync.dma_start(
            x_sb[:, :, 1024:2048],
            x[:, 1024:2048].rearrange("(a p) c -> p a c", a=NCT))
        for part in range(1, 3):
            for ct in range(NCT):
                nc.scalar.dma_start(
                    wq_sb[ct][:, part * HID:(part + 1) * HID],
                    wq[ct * 128:(ct + 1) * 128, part * HID:(part + 1) * HID])

        # idn/wo/bo/g2 are needed only late; ride the gpsimd SWDGE queue.
        idn_sb = const.tile([128, 128], BF16, tag="idn")
        wo_sb = [const.tile([128, C], BF16, tag=f"wo{kt}", name=f"wo{kt}")
                 for kt in range(4)]
        bo_sb = [const.tile([128, 1], F32, tag=f"bo{mt}", name=f"bo{mt}")
                 for mt in range(2)]
        g2_sb = [const.tile([128, 1], F32, tag=f"g2{mt}", name=f"g2{mt}")
                 for mt in range(2)]

        def _late_dmas():
            nc.gpsimd.dma_start(idn_sb[:], idn)

        def _later_dmas():
            for kt in range(4):
                nc.gpsimd.dma_start(wo_sb[kt][:],
                                    wo[kt * 128:(kt + 1) * 128, :])
            for mt in range(2):
                nc.gpsimd.dma_start(bo_sb[mt][:],
                                    bo[mt * 128:(mt + 1) * 128, :])
                nc.gpsimd.dma_start(g2_sb[mt][:],
                                    g2[mt * 128:(mt + 1) * 128, :])

        q_sb, k_sb = [None] * 4, [None] * 4
        vt_sb = [None] * NJ
        y4_sb = []

        with tc.tile_pool(name="ps", bufs=3, space="PSUM") as psp, \
                tc.tile_pool(name="ps_yt", bufs=2, space="PSUM") as ps_yt:

            def pstile(name):
                return psp.tile([128, 1024], F32, tag="sim", name=name)

            # --- PE pstate warmup: zero matmuls chained on one arena tile
            # while the first x chunks stream in, so the real front matmuls
            # run at full clock. Output is never read. ---
            dmy_ps = pstile("dmy")
            for i in range(7):
                nc.tensor.matmul(dmy_ps[:8, 0:512], ones128[:, 0:8],
                                 dmyr[:], start=True, stop=True)

            # --- chunked input rms-norm: ss (PE bcast-reduce, rows arrive
            # pre-broadcast) -> ln (ACT) -> exp(-.5) (ACT) = rsqrt -> s_bc
            # (row-broadcast, for the per-column q/k scale-folding). Own
            # half (A) in two 512 chunks for a lean chain; far half (B) at
            # 1024 wide. A second, transposed copy s_t [128, NJ] comes from
            # 1-column reduces (out[p, 0] = ss[jt*128+p]) + a tiny ln/exp;
            # it feeds the vT conversions as a per-PARTITION scale, so vT
            # projects from raw x and xn never exists.
            s_bcA = sbnp.tile([128, 1024], F32, tag="sbcA", name="s_bcA")
            s_bcB = sbnp.tile([128, 1024], F32, tag="sbcB", name="s_bcB")
            sq8a = sbnp.tile([128, NCT, 1024], BF16, tag="sq8", name="sq8a")
            for c in range(2):
                cs = slice(c * 512, (c + 1) * 512)
                nc.vector.tensor_mul(sq8a[:, :, cs], x_sb[:, :, cs],
                                     x_sb[:, :, cs])
                ssps = ps_yt.tile([128, 512], F32, tag="yt", name=f"ss{c}")
                for ct in range(NCT):
                    nc.tensor.matmul(ssps[:], ones128[:],
                                     sq8a[:, ct, cs],
                                     start=(ct == 0), stop=(ct == NCT - 1))
                s_ln = slnp.tile([128, 512], F32, tag="sln", name=f"sln{c}")
                nc.scalar.activation(s_ln[:], ssps[:], AF.Ln,
                                     bias=eps_t[:], scale=1.0 / C)
                nc.scalar.activation(s_bcA[:, cs], s_ln[:], AF.Exp,
                                     bias=zero_t[:], scale=-0.5)
            sq8b = sbnp.tile([128, NCT, 1024], BF16, tag="sq8b", name="sq8b")
            nc.vector.tensor_mul(sq8b[:], x_sb[:, :, 1024:2048],
                                 x_sb[:, :, 1024:2048])
            ssb = pstile("ssb")
            for n in range(2):
                for ct in range(NCT):
                    nc.tensor.matmul(ssb[:, n * 512:(n + 1) * 512],
                                     ones128[:],
                                     sq8b[:, ct, n * 512:(n + 1) * 512],
                                     start=(ct == 0), stop=(ct == NCT - 1))
            s_lnb = slnp.tile([128, 1024], F32, tag="slnb", name="slnb")
            nc.scalar.activation(s_lnb[:], ssb[:], AF.Ln,
                                 bias=eps_t[:], scale=1.0 / C)
            nc.scalar.activation(s_bcB[:], s_lnb[:], AF.Exp,
                                 bias=zero_t[:], scale=-0.5)
            # transposed s: one 1-column reduce per key tile. One shared
            # start (pending-zero covers the whole bank region; each
            # column's first write overwrites, the second accumulates).
            sstp = ps_yt.tile([128, NJ], F32, tag="yt", name="sstp")
            for jt in range(NJ):
                sq8v = (sq8a if jt < 8 else sq8b)
                jo = (jt % 8) * 128
                for ct in range(NCT):
                    nc.tensor.matmul(
                        sstp[:, jt:jt + 1],
                        sq8v[:, ct, jo:jo + 128],
                        ones128[:, 0:1],
                        start=(jt == 0 and ct == 0),
                        stop=(jt == NJ - 1 and ct == NCT - 1),
                        skip_group_check=True)
            st_ln = slnp.tile([128, NJ], F32, tag="stln", name="st_ln")
            nc.scalar.activation(st_ln[:], sstp[:], AF.Ln,
                                 bias=eps_t[:], scale=1.0 / C)
            s_t = sbnp.tile([128, NJ], F32, tag="st", name="s_t")
            nc.scalar.activation(s_t[:], st_ln[:], AF.Exp,
                                 bias=zero_t[:], scale=-0.5)

            # --- projection helpers (psum from the shared arena) ---
            # All projections run on RAW x; the norm scale s is applied at
            # the psum->sbuf conversion (mathematically identical since s
            # is per-column). For q/k that's a per-column mul with the
            # row-broadcast s (DVE tensor_tensor); for vT the column index
            # is the PARTITION, so the transposed s_t rides a per-partition
            # scale and either engine can convert.
            def make_q(mt):
                ps = pstile(f"qps{mt}")
                for n in range(2):
                    for ct in range(NCT):
                        nc.tensor.matmul(
                            ps[:, n * 512:(n + 1) * 512],
                            wq_sb[ct][:, mt * 128:(mt + 1) * 128],
                            x_sb[:, ct, n * 512:(n + 1) * 512],
                            start=(ct == 0), stop=(ct == NCT - 1))
                t = qp.tile([128, LQ], BF16, tag="q", name=f"qsb{mt}")
                nc.vector.tensor_mul(t[:], ps[:, :], s_bcA[:])
                q_sb[mt] = t

            def make_k_half(mt, half):
                if half == 0:
                    k_sb[mt] = kp.tile([128, L], BF16, tag="k",
                                       name=f"ksb{mt}")
                t = k_sb[mt]
                ps = pstile(f"kps{mt}_{half}")
                for n in range(2):
                    for ct in range(NCT):
                        nc.tensor.matmul(
                            ps[:, n * 512:(n + 1) * 512],
                            wq_sb[ct][:, HID + mt * 128:HID + (mt + 1) * 128],
                            x_sb[:, ct,
                                 half * 1024 + n * 512:
                                 half * 1024 + (n + 1) * 512],
                            start=(ct == 0), stop=(ct == NCT - 1))
                nc.vector.tensor_mul(t[:, half * LQ:(half + 1) * LQ],
                                     ps[:, :],
                                     (s_bcA if half == 0 else s_bcB)[:])

            def make_vt(jt, eng):
                ps = pstile(f"vps{jt}")
                for ct in range(NCT):
                    nc.tensor.matmul(ps[:, 0:512],
                                     x_sb[:, ct, jt * 128:(jt + 1) * 128],
                                     wq_sb[ct][:, 2 * HID:3 * HID],
                                     start=(ct == 0), stop=(ct == NCT - 1))
                t = vtp.tile([128, H, D + 1], BF16, tag="vt", name=f"vt{jt}")
                src = ps[:, 0:512].rearrange("p (h e) -> p h e", e=D)
                if eng is nc.scalar:
                    eng.activation(t[:, :, 0:D], src, AF.Identity,
                                   bias=zero_t[:], scale=s_t[:, jt:jt + 1])
                else:
                    eng.tensor_scalar_mul(t[:, :, 0:D], src,
                                          s_t[:, jt:jt + 1])
                nc.gpsimd.tensor_copy(
                    t[:, :, D:D + 1].rearrange("p h o -> p (h o)"),
                    stage[:, 0:H])
                vt_sb[jt] = t

            # Minimal prefix for sim h0: q mt0, k mt0 both halves; they
            # depend only on x + wq + the s chain. vT tiles are emitted
            # inside h0's slots.
            make_q(0)
            make_k_half(0, 0)
            make_k_half(0, 1)

            # --- attention: head h sims + exp stream while head h-1's yT
            # accumulates from its fully-materialized pt tile; remaining
            # projections interleave into h0-h3's slots. ---
            pt_sb = [None] * H
            yts_sb = [None] * 4
            out_ps = []
            state = {}

            def sim_chunk(h, jt):
                mt, po = h // 2, (h % 2) * D
                sps = pstile(f"sps{h}_{jt}")
                for n in range(LQ // 512):
                    nc.tensor.matmul(
                        sps[:, n * 512:(n + 1) * 512],
                        k_sb[mt][po:po + D, jt * 128:(jt + 1) * 128],
                        q_sb[mt][po:po + D, n * 512:(n + 1) * 512],
                        start=True, stop=True)
                dst = pt_sb[h][:, jt, :]
                if EXP_ENG[h][jt] == 'V':
                    nc.vector.tensor_scalar(
                        dst.bitcast(U16), sps[:, :], SCH_A, SCH_B,
                        op0=MUL, op1=ADD)
                else:
                    nc.scalar.activation(dst, sps[:, :], AF.Exp,
                                         bias=zero_t[:], scale=0.125)

            # yT accumulation runs lag-~5-jt behind the SAME head's sim/exp
            # stream (jt-major groups make this legal), so only the last
            # jt-group spills past the head's final sim -- the old
            # one-full-head lag put the whole last yT serially in the tail.
            def yt_alloc(h):
                y0 = ps_yt.tile([128, 4, D + 1], F32, tag="yt",
                                name=f"yt{h}_0")
                y1 = ps_yt.tile([128, 4, D + 1], F32, tag="yt",
                                name=f"yt{h}_1")
                state[h] = {"yps": [y0, y1]}

            def yt_unit(h, g, half):
                """16 matmuls: jt-group 4g..4g+3 x 4 query-chunks into the
                `half` accumulator."""
                yps, pt = state[h]["yps"][half], pt_sb[h]
                for ic in range(4):
                    for jt in range(4 * g, 4 * g + 4):
                        nc.tensor.matmul(
                            yps[:, ic, :],
                            pt[:, jt, (half * 4 + ic) * 128:
                               (half * 4 + ic + 1) * 128],
                            vt_sb[jt][:, h, :],
                            start=(g == 0 and ic == 0 and jt == 0),
                            stop=(jt == NJ - 1),
                            skip_group_check=True)

            def yt_finish(h, half):
                yps = state[h]["yps"][half]
                po = (h % 2) * D
                if half == 0:
                    inv = invp.tile([128, 8], F32, tag="inv",
                                    name=f"inv{h}")
                    state[h]["inv"] = inv
                else:
                    inv = state[h]["inv"]
                nc.vector.reciprocal(
                    inv[:, half * 4:(half + 1) * 4],
                    yps[:, :, D:D + 1].rearrange("p a b -> p (a b)"))
                nc.vector.tensor_mul(
                    yts_sb[h // 2][:, half * 4:(half + 1) * 4, po:po + D],
                    yps[:, :, 0:D],
                    inv[:, half * 4:(half + 1) * 4].unsqueeze(2)
                    .broadcast_to((128, 4, D)))

            def transpose_pair(pair):
                rt = pstile(f"y4ps{pair}")
                y4ps = rt[:].bitcast(BF16)[:, 0:LQ]
                for ich in range(NI):
                    nc.tensor.transpose(y4ps[:, ich * 128:(ich + 1) * 128],
                                        yts_sb[pair][:, ich, :], idn_sb[:])
                y4 = y4p.tile([128, LQ], BF16, tag="y4", name=f"y4_{pair}")
                nc.scalar.copy(y4[:], y4ps)
                y4_sb.append(y4)

            # slot plan: remaining projections and conversions placed into
            # specific (h, jt) slots so their psum use and conv engine time
            # hide under the sim/exp/yt stream. vt emission must stay ahead
            # of the same head's yt units (group g needs vt 4g..4g+3) but
            # behind its Pool xn chunk so PE wait-queue parking is short.
            H0_VT = {1: 0, 2: 1, 3: 2, 4: 3, 5: 4, 6: 5, 8: 6, 9: 7,
                     10: 8, 11: 9, 12: 10, 13: 11, 14: 12, 15: 13}

            VT_V = frozenset((6, 7, 10, 11, 14, 15))

            def slot(h, jt):
                if h == 0:
                    if jt in H0_VT:
                        v = H0_VT[jt]
                        make_vt(v, nc.vector if v in VT_V else nc.scalar)
                elif h == 1:
                    if jt == 0:
                        make_vt(14, nc.vector)
                        make_vt(15, nc.scalar)
                    elif jt == 4:
                        _late_dmas()
                    elif jt == 6:
                        make_q(1)
                    elif jt == 10:
                        make_k_half(1, 0)
                    elif jt == 12:
                        make_k_half(1, 1)
                elif h == 2:
                    if jt == 2:
                        make_q(2)
                    elif jt == 6:
                        make_k_half(2, 0)
                    elif jt == 10:
                        make_k_half(2, 1)
                elif h == 3:
                    if jt == 2:
                        make_q(3)
                    elif jt == 6:
                        make_k_half(3, 0)
                    elif jt == 10:
                        make_k_half(3, 1)
                elif h == 5 and jt == 2:
                    _later_dmas()

            def out_pair_partial(mt):
                ops = pstile(f"ops{mt}")
                out_ps.append(ops)
                for pair in range(3):
                    for n in range(LQ // 512):
                        nc.tensor.matmul(
                            ops[:, n * 512:(n + 1) * 512],
                            wo_sb[pair][:, mt * 128:(mt + 1) * 128],
                            y4_sb[pair][:, n * 512:(n + 1) * 512],
                            start=(pair == 0), stop=False)

            # pair-3 transpose is split into two arena tiles so the first
            # half's slot frees as soon as its conv is done (the tail's
            # psum ring is the serializer otherwise).
            t3 = {}

            def transpose3_half(half):
                if half == 0:
                    t3["y4"] = y4p.tile([128, LQ], BF16, tag="y4",
                                        name="y4_3")
                    y4_sb.append(t3["y4"])
                rt = pstile(f"y4ps3_{half}")
                y4ps = rt[:].bitcast(BF16)[:, 0:512]
                for i in range(4):
                    ich = half * 4 + i
                    nc.tensor.transpose(y4ps[:, i * 128:(i + 1) * 128],
                                        yts_sb[3][:, ich, :], idn_sb[:])
                hs = slice(half * 512, (half + 1) * 512)
                nc.scalar.copy(t3["y4"][:, hs], y4ps)

            YT_UNITS = {7: (0, 0), 8: (0, 1), 11: (1, 0), 12: (1, 1),
                        15: (2, 0)}
            YT_SPILL = {0: (2, 1), 1: (3, 0), 2: (3, 1)}
            for h in range(H + 1):
                if h < H:
                    pt_sb[h] = ptp.tile([128, NJ, LQ], BF16, tag="pt",
                                        name=f"pt{h}")
                    if h % 2 == 0:
                        yts_sb[h // 2] = ytsp.tile([128, NI, 128], BF16,
                                                   tag="yts",
                                                   name=f"yts{h // 2}")
                for jt in range(NJ):
                    if h < H:
                        sim_chunk(h, jt)
                    slot(h, jt)
                    if h < H:
                        if jt == 6:
                            yt_alloc(h)
                        elif jt in YT_UNITS:
                            yt_unit(h, *YT_UNITS[jt])
                    if h > 0 and jt in YT_SPILL:
                        g, half = YT_SPILL[jt]
                        yt_unit(h - 1, g, half)
                        if g == 3:
                            yt_finish(h - 1, half)
                    if jt == 8 and h in (2, 4, 6):
                        transpose_pair(h // 2 - 1)
                    if h == H:
                        # out-proj partials first: they depend only on the
                        # long-ready pairs 0-2, so they must not queue
                        # behind the pair-3 transposes in the PE sequencer.
                        if jt == 2:
                            out_pair_partial(0)
                        elif jt == 3:
                            out_pair_partial(1)
                        elif jt == 4:
                            transpose3_half(0)
                        elif jt == 5:
                            transpose3_half(1)

            # ------- pipelined out-proj tail: per 256-col block, finish the
            # pair-3 accumulation, bias, sum-sq, ln/exp rsqrt, scale, store.
            out_sb = outp.tile([128, 2, LQ], BF16, tag="osb")
            # ss2 accumulators live in the yT psum banks (free by now);
            # block pairs alternate between the two so a block's start=True
            # bank reset never waits on the previous block's Ln read.
            ss2ps = [ps_yt.tile([128, 2, 256], F32, tag="yt",
                                name=f"ss2ps{i}") for i in range(2)]
            for nb in range(4):
                bs = slice(nb * 256, (nb + 1) * 256)
                ssv = ss2ps[nb % 2][:, nb // 2, :]
                for mt in range(2):
                    nc.tensor.matmul(
                        out_ps[mt][:, bs],
                        wo_sb[3][:, mt * 128:(mt + 1) * 128],
                        t3["y4"][:, bs],
                        start=False, stop=True, skip_group_check=True)
                nc.vector.tensor_scalar_add(out_sb[:, 0, bs],
                                            out_ps[0][:, bs], bo_sb[0])
                nc.scalar.activation(out_sb[:, 1, bs], out_ps[1][:, bs],
                                     AF.Identity, bias=bo_sb[1][:],
                                     scale=1.0)
                sq2b = sqp.tile([128, 2, 256], BF16, tag="sq2",
                                name=f"sq2_{nb}")
                nc.vector.tensor_mul(sq2b[:], out_sb[:, :, bs],
                                     out_sb[:, :, bs])
                for mt in range(2):
                    nc.tensor.matmul(ssv, ones128[:], sq2b[:, mt, :],
                                     start=(mt == 0), stop=(mt == 1))
                s2ln = slnp.tile([128, 256], F32, tag="sln",
                                 name=f"s2ln{nb}")
                nc.scalar.activation(s2ln[:], ssv, AF.Ln,
                                     bias=eps_t[:], scale=1.0 / C)
                s2bc = sbcp.tile([128, 256], F32, tag="sbc",
                                 name=f"s2bc{nb}")
                nc.scalar.activation(s2bc[:], s2ln[:], AF.Exp,
                                     bias=zero_t[:], scale=-0.5)
                if nb % 2 == 0:
                    fin = finp.tile([128, 2, 512], F32, tag="fin",
                                    name=f"fin{nb}")
                for mt in range(2):
                    nc.vector.scalar_tensor_tensor(
                        fin[:, mt, (nb % 2) * 256:(nb % 2) * 256 + 256],
                        out_sb[:, mt, bs], g2_sb[mt][:],
                        s2bc[:], op0=MUL, op1=MUL)
                if nb % 2 == 1:
                    hs = slice((nb - 1) * 256, (nb + 1) * 256)
                    for mt in range(2):
                        nc.sync.dma_start(out[mt * 128:(mt + 1) * 128, hs],
                                          fin[:, mt, :])


_NC = None


def _get_nc():
    global _NC
    if _NC is None:
        nc = bacc.Bacc("TRN2", target_bir_lowering=False, debug=False,
                       enable_asserts=False, num_devices=8)
        x_d = nc.dram_tensor("x", [C, L], BF16, kind="ExternalInput")
        wq_d = nc.dram_tensor("wqkvT", [C, 3 * HID], BF16, kind="ExternalInput")
        wo_d = nc.dram_tensor("woutT", [HID, C], BF16, kind="ExternalInput")
        b_d = nc.dram_tensor("bout", [C, 1], F32, kind="ExternalInput")
        g2_d = nc.dram_tensor("g2v", [C, 1], F32, kind="ExternalInput")
        idn_d = nc.dram_tensor("idn", [128, 128], BF16, kind="ExternalInput")
        out_d = nc.dram_tensor("out", [C, LQ], F32, kind="ExternalOutput")
        with tile.TileContext(nc) as tc:
            _body(tc, x_d.ap(), wq_d.ap(), wo_d.ap(), b_d.ap(), g2_d.ap(),
                  idn_d.ap(), out_d.ap())
        nc.compile()
        _NC = nc
    return _NC


def _in_maps(x, g1, w_qkv, w_out, b_out, g2):
    BFH = ml_dtypes.bfloat16
    w2 = (np.asarray(w_qkv, np.float32)
          * np.asarray(g1, np.float32).reshape(1, C))
    wqkvT = np.ascontiguousarray(w2.T).astype(BFH)
    woutT = np.ascontiguousarray(np.asarray(w_out, np.float32).T).astype(BFH)
    bo = np.asarray(b_out, np.float32).reshape(C, 1)
    g2v = np.asarray(g2, np.float32).reshape(C, 1)
    idn = np.eye(128, dtype=BFH)
    maps = []
    for core in range(8):
        b, half = divmod(core, 2)
        xb = np.asarray(x[b], np.float32)
        x_core = np.ascontiguousarray(np.concatenate(
            [xb[:, half * LQ:(half + 1) * LQ],
             xb[:, (1 - half) * LQ:(2 - half) * LQ]], axis=1)).astype(BFH)
        maps.append({"x": x_core, "wqkvT": wqkvT, "woutT": woutT,
                     "bout": bo, "g2v": g2v, "idn": idn})
    return maps


def _assemble(results):
    out = np.empty((B, C, L), np.float32)
    for core in range(8):
        b, half = divmod(core, 2)
        out[b][:, half * LQ:(half + 1) * LQ] = results[core]["out"]
    return out


def kernel(x, g1, w_qkv, w_out, b_out, g2, _trace=False, _tmpdir=None):
    res = run_bass_kernel_spmd(_get_nc(),
                               _in_maps(x, g1, w_qkv, w_out, b_out, g2),
                               core_ids=list(range(8)), trace=_trace,
                               tmpdir=_tmpdir)
    out = _assemble(res.results)
    if _trace:
        return out, res
    return out


# revision 85
# speedup vs baseline: 1.0279x; 1.0279x over previous
"""Trainium2 Bass kernel for the attention module (b=4, c=256, l=2048, h=8, d=64).

Sharding: 8 cores = 4 batches x 2 query-halves (no collectives). Each core
receives its batch's x with columns permuted so its own query half comes
first; it computes k/v for all 2048 key positions and the attention output
for its 1024 queries, then the output projection + final rms-norm.

Device algorithm per core (all matmul data bf16, psum fp32):
  ss     = ones128.T @ bf16(x*x)            (PE bcast-reduce: [128,*] rows
                                             all equal to column sum-sq)
  s      = exp(-0.5*ln(ss/256+eps))         (ACT Ln + ACT Exp: rsqrt using
                                             the single natural_log_exp
                                             table -- no mid-kernel ACT
                                             table swaps, ever)
  xn     = bf16(x * s)                      (DVE, s pre-broadcast)
  q,k    = bf16(wqkvT-slices.T @ xn)        (g1 folded on host)
  vT     = bf16(xn_slice.T @ wv)
  per head:  simT[j,i] = k_h.T @ q_h        (psum fp32, logits*8)
             pt = exp(simT/8) as bf16       (split across ACT true exp, DVE
                                             Schraudolph bitcast-exp, and a
                                             DMA->SBUF + Pool Schraudolph
                                             path so no engine gates PE)
             ytps[i, d] += pt_chunk.T @ vT_h
  y_h    = ytps[:, 0:64] * (1/den) broadcast (DVE, stride-0 bcast; den
                                             rides yt as a fused ones col)
  y4     = PE transpose(yts, identity)      (bf16, per head-pair)
  out    = woT.T @ y4 + b_out
  result = out * g2 * exp(-0.5*ln(ss2/256+eps))   (bcast-reduce again)

Scheduling: 512-col-chunked input norm so the first q/k projections start
~4us in; one rotating 3x[128,1024] psum arena for projections, sim chunks,
pair transposes and the out-proj; head h's sim+exp stream overlaps head
h-1's yT accumulation; 256-col-pipelined output tail (out-proj pair-3,
bias, sum-sq, ln/exp, scale, store per block) to cut the serial tail.
Late-consumed tensors (idn/wo/bo/g2) are DMA'd on the gpsimd SWDGE queue.
"""
import sys

import numpy as np

if "/opt/trn_rl_repo" not in sys.path:
    sys.path.insert(0, "/opt/trn_rl_repo")

import ml_dtypes  # noqa: E402

import concourse.bass as bass  # noqa: E402
import concourse.tile as tile  # noqa: E402
from concourse import bacc, mybir  # noqa: E402
from concourse.bass_utils import run_bass_kernel_spmd  # noqa: E402

F32 = mybir.dt.float32
F32R = mybir.dt.float32r
BF16 = mybir.dt.bfloat16
U16 = mybir.dt.uint16
AF = mybir.ActivationFunctionType
MUL = mybir.AluOpType.mult
ADD = mybir.AluOpType.add

B, C, L = 4, 256, 2048
H, D = 8, 64
HID = H * D
LQ = L // 2      # queries per core
NCT = C // 128   # 2 c-tiles
NJ = L // 128    # 16 j-tiles
NI = LQ // 128   # 8 i-chunks
EPS_B = 1e-26

LOG2E = 1.4426950408889634
# Schraudolph bf16: bits = round(logit*128*log2e + 127*128 - 5.6); logit = sim/8
SCH_A = 128.0 * LOG2E / 8.0
SCH_B = 127.0 * 128.0 - 5.6

# exp engine split per (h, jt): 'A' ACT true exp, 'V' DVE Schraudolph.
# (PSUM cannot be DMA'd and Pool cannot read PSUM, so these two engines
# carry all 128 exp tiles; the ratio balances their total load just under
# the PE roofline.) Alternation keeps the 3-deep sim psum ring draining
# on both engines at once.
_ROW8A = ['A', 'V', 'A', 'V', 'A', 'V', 'A', 'V',
          'A', 'V', 'A', 'V', 'A', 'V', 'A', 'V']
_ROW9A = ['A', 'V', 'A', 'V', 'A', 'A', 'V', 'A',
          'V', 'A', 'A', 'V', 'A', 'V', 'A', 'A']
# h0's DVE is busy with the s-folded q/k conversions until ~jt6, so its
# early exps go to ACT.
_ROWH0 = ['A', 'A', 'A', 'A', 'A', 'V', 'A', 'V',
          'A', 'V', 'A', 'V', 'A', 'V', 'A', 'V']
_ROWH7 = ['A', 'V', 'A', 'V', 'A', 'A', 'V', 'A',
          'V', 'A', 'A', 'V', 'A', 'V', 'S', 'S']
EXP_ENG = ([list(_ROWH0)]
           + [list(_ROW9A) if h % 2 else list(_ROW8A)
              for h in range(1, H - 1)]
           + [list(_ROWH7)])


def _body(tc, x, wq, wo, bo, g2, idn, out):
    nc = tc.nc
    from contextlib import ExitStack
    with ExitStack() as ctx:
        ctx.enter_context(nc.allow_low_precision(
            reason="bf16 data path by design"))
        const = ctx.enter_context(tc.tile_pool(name="const", bufs=1))
        bigx = ctx.enter_context(tc.tile_pool(name="bigx", bufs=1))
        sqp = ctx.enter_context(tc.tile_pool(name="sq", bufs=2))
        slnp = ctx.enter_context(tc.tile_pool(name="sln", bufs=2))
        sbnp = ctx.enter_context(tc.tile_pool(name="sbn", bufs=1))
        sbcp = ctx.enter_context(tc.tile_pool(name="sbc", bufs=4))
        qp = ctx.enter_context(tc.tile_pool(name="q", bufs=4))
        kp = ctx.enter_context(tc.tile_pool(name="k", bufs=4))
        vtp = ctx.enter_context(tc.tile_pool(name="vt", bufs=16))
        ptp = ctx.enter_context(tc.tile_pool(name="pt", bufs=2))
        invp = ctx.enter_context(tc.tile_pool(name="inv", bufs=2))
        ytsp = ctx.enter_context(tc.tile_pool(name="yts", bufs=2))
        y4p = ctx.enter_context(tc.tile_pool(name="y4", bufs=4))
        outp = ctx.enter_context(tc.tile_pool(name="outp", bufs=1))
        finp = ctx.enter_context(tc.tile_pool(name="fin", bufs=2))

        # ---------------- constants ----------------
        # Preload the one ACT table that covers every function this kernel
        # uses (Ln, Exp, Copy, Identity). Bacc's auto-inserter is greedy
        # per-function and would thrash 13 loads between the single-function
        # tables; with this explicit load it sees every activation covered.
        from concourse.hw_specs import get_activation_tables
        tabs = list(get_activation_tables(nc.m.arch))
        joint_id = tabs.index("natural_log_exp_and_others")
        nc.scalar.add_instruction(mybir.InstLoadActFuncSet(
            name=nc.get_next_instruction_name(),
            act_func_set_id=joint_id, engine=mybir.EngineType.Activation,
            ins=[], outs=[]))

        dmyr = const.tile([128, 512], BF16, tag="dmyr")
        nc.gpsimd.memset(dmyr[:], 0.0)
        ones128 = const.tile([128, 128], BF16, tag="ones128")
        nc.vector.memset(ones128[:], 1.0)
        ones_row = const.tile([1, 512], BF16, tag="ones_row")
        nc.vector.memset(ones_row[:], 1.0)
        stage = const.tile([128, 8], F32, tag="stage")
        nc.gpsimd.memset(stage[:], 1.0)
        eps_t = const.tile([128, 1], F32, tag="eps")
        nc.gpsimd.memset(eps_t[:], EPS_B)
        zero_t = const.tile([128, 1], F32, tag="zero")
        nc.gpsimd.memset(zero_t[:], 0.0)

        # ---------------- input DMAs ----------------
        # x as four 512-col chunk DMAs, each moving BOTH c-tiles in one
        # transfer (256 dram rows -> [128, 2, 512] sbuf) so only 4 HWDGE
        # setups serialize. wq rides the DVE queue so the ACT sequencer
        # stays free for the norm-chain Ln/Exp.
        x_sb = bigx.tile([128, NCT, L], BF16, tag="x")
        wq_sb = []
        for ct in range(NCT):
            wq_sb.append(const.tile([128, 3 * HID], BF16, tag=f"wq{ct}",
                                    name=f"wq{ct}"))
        nc.sync.dma_start(
            x_sb[:, :, 0:1024],
            x[:, 0:1024].rearrange("(a p) c -> p a c", a=NCT))
        for ct in range(NCT):
            nc.sync.dma_start(wq_sb[ct][:, 0:HID],
                              wq[ct * 128:(ct + 1) * 128, 0:HID])
        nc.sync.dma_start(
            x_sb[:, :, 1024:2048],
            x[:, 1024:2048].rearrange("(a p) c -> p a c", a=NCT))
        for part in range(1, 3):
            for ct in range(NCT):
                nc.sync.dma_start(
                    wq_sb[ct][:, part * HID:(part + 1) * HID],
                    wq[ct * 128:(ct + 1) * 128, part * HID:(part + 1) * HID])

        # idn/wo/bo/g2 are needed only late; ride the gpsimd SWDGE queue.
        idn_sb = const.tile([128, 128], BF16, tag="idn")
        wo_sb = [const.tile([128, C], BF16, tag=f"wo{kt}", name=f"wo{kt}")
                 for kt in range(4)]
        bo_row = const.tile([1, C], BF16, tag="bo_row")
        g2_sb = [const.tile([128, 1], F32, tag=f"g2{mt}", name=f"g2{mt}")
                 for mt in range(2)]

        def _late_dmas():
            nc.gpsimd.dma_start(idn_sb[:], idn)

        def _later_dmas():
            for kt in range(4):
                nc.gpsimd.dma_start(wo_sb[kt][:],
                                    wo[kt * 128:(kt + 1) * 128, :])
            nc.gpsimd.dma_start(bo_row[:], bo)
            for mt in range(2):
                nc.gpsimd.dma_start(g2_sb[mt][:],
                                    g2[mt * 128:(mt + 1) * 128, :])

        q_sb, k_sb = [None] * 4, [None] * 4
        vt_sb = [None] * NJ
        y4_sb = []

        with tc.tile_pool(name="ps", bufs=3, space="PSUM") as psp, \
                tc.tile_pool(name="ps_yt", bufs=2, space="PSUM") as ps_yt:

            def pstile(name):
                return psp.tile([128, 1024], F32, tag="sim", name=name)

            # --- PE pstate warmup: zero matmuls chained on one arena tile
            # while the first x chunks stream in, so the real front matmuls
            # run at full clock. Output is never read. ---
            dmy_ps = pstile("dmy")
            for i in range(7):
                nc.tensor.matmul(dmy_ps[:8, 0:512], ones128[:, 0:8],
                                 dmyr[:], start=True, stop=True)

            # --- chunked input rms-norm: ss (PE bcast-reduce, rows arrive
            # pre-broadcast) -> ln (ACT) -> exp(-.5) (ACT) = rsqrt -> s_bc
            # (row-broadcast, for the per-column q/k scale-folding). Own
            # half (A) in two 512 chunks for a lean chain; far half (B) at
            # 1024 wide. A second, transposed copy s_t [128, NJ] comes from
            # 1-column reduces (out[p, 0] = ss[jt*128+p]) + a tiny ln/exp;
            # it feeds the vT conversions as a per-PARTITION scale, so vT
            # projects from raw x and xn never exists.
            hp = ctx.enter_context(tc.high_priority())
            s_bcA = sbnp.tile([128, 1024], F32, tag="sbcA", name="s_bcA")
            sq8a = sbnp.tile([128, NCT, 1024], BF16, tag="sq8", name="sq8a")
            for c in range(2):
                cs = slice(c * 512, (c + 1) * 512)
                nc.vector.tensor_mul(sq8a[:, :, cs], x_sb[:, :, cs],
                                     x_sb[:, :, cs])
                ssps = ps_yt.tile([128, 512], F32, tag="yt", name=f"ss{c}")
                for ct in range(NCT):
                    nc.tensor.matmul(ssps[:], ones128[:],
                                     sq8a[:, ct, cs],
                                     start=(ct == 0), stop=(ct == NCT - 1))
                s_ln = slnp.tile([128, 512], F32, tag="sln", name=f"sln{c}")
                nc.scalar.activation(s_ln[:], ssps[:], AF.Ln,
                                     bias=eps_t[:], scale=1.0 / C)
                nc.scalar.activation(s_bcA[:, cs], s_ln[:], AF.Exp,
                                     bias=zero_t[:], scale=-0.5)
            sq8b = sbnp.tile([128, NCT, 1024], BF16, tag="sq8b", name="sq8b")
            nc.vector.tensor_mul(sq8b[:], x_sb[:, :, 1024:2048],
                                 x_sb[:, :, 1024:2048])
            # transposed s: one 1-column reduce per key tile. One shared
            # start (pending-zero covers the whole bank region; each
            # column's first write overwrites, the second accumulates).
            sstp = ps_yt.tile([128, NJ], F32, tag="yt", name="sstp")
            for jt in range(NJ):
                sq8v = (sq8a if jt < 8 else sq8b)
                jo = (jt % 8) * 128
                for ct in range(NCT):
                    nc.tensor.matmul(
                        sstp[:, jt:jt + 1],
                        sq8v[:, ct, jo:jo + 128],
                        ones128[:, 0:1],
                        start=(jt == 0 and ct == 0),
                        stop=(jt == NJ - 1 and ct == NCT - 1),
                        skip_group_check=True)
            st_ln = slnp.tile([128, NJ], F32, tag="stln", name="st_ln")
            nc.scalar.activation(st_ln[:], sstp[:], AF.Ln,
                                 bias=eps_t[:], scale=1.0 / C)
            s_t = sbnp.tile([128, NJ], F32, tag="st", name="s_t")
            nc.scalar.activation(s_t[:], st_ln[:], AF.Exp,
                                 bias=zero_t[:], scale=-0.5)
            # key-side norm scale folded into the exp: per-partition scale
            # APs (sim partitions ARE keys), pre-multiplied by the logit
            # scales of the two exp flavors.
            s_t8 = sbnp.tile([128, NJ], F32, tag="st8", name="s_t8")
            nc.vector.tensor_scalar_mul(s_t8[:], s_t[:], 0.125)
            s_tA = sbnp.tile([128, NJ], F32, tag="stA", name="s_tA")
            nc.vector.tensor_scalar_mul(s_tA[:], s_t[:], SCH_A)
            ctx.pop_all().close() if False else None

            # --- projection helpers (psum from the shared arena) ---
            # All projections run on RAW x; the norm scale s is applied at
            # the psum->sbuf conversion (mathematically identical since s
            # is per-column). For q/k that's a per-column mul with the
            # row-broadcast s (DVE tensor_tensor); for vT the column index
            # is the PARTITION, so the transposed s_t rides a per-partition
            # scale and either engine can convert.
            def make_q(mt, split=False):
                ps = pstile(f"qps{mt}")
                for n in range(2):
                    for ct in range(NCT):
                        nc.tensor.matmul(
                            ps[:, n * 512:(n + 1) * 512],
                            wq_sb[ct][:, mt * 128:(mt + 1) * 128],
                            x_sb[:, ct, n * 512:(n + 1) * 512],
                            start=(ct == 0), stop=(ct == NCT - 1))
                t = qp.tile([128, LQ], BF16, tag="q", name=f"qsb{mt}")
                if split:
                    for c in range(2):
                        cs = slice(c * 512, (c + 1) * 512)
                        nc.vector.tensor_mul(t[:, cs], ps[:, cs],
                                             s_bcA[:, cs])
                else:
                    nc.vector.tensor_mul(t[:], ps[:, :], s_bcA[:])
                q_sb[mt] = t

            def make_k_half(mt, half, eng=None):
                if half == 0:
                    k_sb[mt] = kp.tile([128, L], BF16, tag="k",
                                       name=f"ksb{mt}")
                t = k_sb[mt]
                ps = pstile(f"kps{mt}_{half}")
                for n in range(2):
                    for ct in range(NCT):
                        nc.tensor.matmul(
                            ps[:, n * 512:(n + 1) * 512],
                            wq_sb[ct][:, HID + mt * 128:HID + (mt + 1) * 128],
                            x_sb[:, ct,
                                 half * 1024 + n * 512:
                                 half * 1024 + (n + 1) * 512],
                            start=(ct == 0), stop=(ct == NCT - 1))
                if eng is nc.scalar:
                    eng.copy(t[:, half * LQ:(half + 1) * LQ], ps[:, :])
                else:
                    nc.vector.tensor_copy(t[:, half * LQ:(half + 1) * LQ],
                                          ps[:, :])

            def make_vt(jt, eng):
                ps = pstile(f"vps{jt}")
                for ct in range(NCT):
                    nc.tensor.matmul(ps[:, 0:512],
                                     x_sb[:, ct, jt * 128:(jt + 1) * 128],
                                     wq_sb[ct][:, 2 * HID:3 * HID],
                                     start=(ct == 0), stop=(ct == NCT - 1))
                t = vtp.tile([128, H, D + 1], BF16, tag="vt", name=f"vt{jt}")
                src = ps[:, 0:512].rearrange("p (h e) -> p h e", e=D)
                if eng is nc.scalar:
                    eng.activation(t[:, :, 0:D], src, AF.Identity,
                                   bias=zero_t[:], scale=s_t[:, jt:jt + 1])
                else:
                    eng.tensor_scalar_mul(t[:, :, 0:D], src,
                                          s_t[:, jt:jt + 1])
                nc.gpsimd.tensor_copy(
                    t[:, :, D:D + 1].rearrange("p h o -> p (h o)"),
                    stage[:, 0:H])
                vt_sb[jt] = t

            # Minimal prefix for sim h0: q mt0, k mt0 both halves; they
            # depend only on x + wq + the s chain. vT tiles are emitted
            # inside h0's slots.
            make_q(0, split=True)
            make_k_half(0, 0, nc.scalar)
            make_k_half(0, 1, nc.scalar)

            # --- attention: head h sims + exp stream while head h-1's yT
            # accumulates from its fully-materialized pt tile; remaining
            # projections interleave into h0-h3's slots. ---
            pt_sb = [None] * H
            yts_sb = [None] * 4
            out_ps = []
            state = {}

            def sim_chunk(h, jt):
                mt, po = h // 2, (h % 2) * D
                sps = pstile(f"sps{h}_{jt}")
                for n in range(LQ // 512):
                    nc.tensor.matmul(
                        sps[:, n * 512:(n + 1) * 512],
                        k_sb[mt][po:po + D, jt * 128:(jt + 1) * 128],
                        q_sb[mt][po:po + D, n * 512:(n + 1) * 512],
                        start=True, stop=True)
                dst = pt_sb[h][:, jt, :]
                code = EXP_ENG[h][jt]
                stA = s_tA[:, jt:jt + 1]
                st8 = s_t8[:, jt:jt + 1]
                if code == 'V':
                    nc.vector.tensor_scalar(
                        dst.bitcast(U16), sps[:, :], stA, SCH_B,
                        op0=MUL, op1=ADD)
                elif code == 'S':
                    nc.scalar.activation(dst[:, 0:512], sps[:, 0:512],
                                         AF.Exp, bias=zero_t[:], scale=st8)
                    nc.vector.tensor_scalar(
                        dst[:, 512:1024].bitcast(U16), sps[:, 512:1024],
                        stA, SCH_B, op0=MUL, op1=ADD)
                else:
                    nc.scalar.activation(dst, sps[:, :], AF.Exp,
                                         bias=zero_t[:], scale=st8)

            # yT accumulation runs lag-~5-jt behind the SAME head's sim/exp
            # stream (jt-major groups make this legal), so only the last
            # jt-group spills past the head's final sim -- the old
            # one-full-head lag put the whole last yT serially in the tail.
            def yt_alloc(h):
                y0 = ps_yt.tile([128, 4, D + 1], F32, tag="yt",
                                name=f"yt{h}_0")
                y1 = ps_yt.tile([128, 4, D + 1], F32, tag="yt",
                                name=f"yt{h}_1")
                state[h] = {"yps": [y0, y1]}

            def yt_unit(h, g, half):
                """16 matmuls: jt-group 4g..4g+3 x 4 query-chunks into the
                `half` accumulator."""
                yps, pt = state[h]["yps"][half], pt_sb[h]
                for ic in range(4):
                    for jt in range(4 * g, 4 * g + 4):
                        nc.tensor.matmul(
                            yps[:, ic, :],
                            pt[:, jt, (half * 4 + ic) * 128:
                               (half * 4 + ic + 1) * 128],
                            vt_sb[jt][:, h, :],
                            start=(g == 0 and ic == 0 and jt == 0),
                            stop=(jt == NJ - 1),
                            skip_group_check=True)

            def yt_finish(h, half):
                yps = state[h]["yps"][half]
                po = (h % 2) * D
                if half == 0:
                    inv = invp.tile([128, 8], F32, tag="inv",
                                    name=f"inv{h}")
                    state[h]["inv"] = inv
                else:
                    inv = state[h]["inv"]
                nc.vector.reciprocal(
                    inv[:, half * 4:(half + 1) * 4],
                    yps[:, :, D:D + 1].rearrange("p a b -> p (a b)"))
                nc.vector.tensor_mul(
                    yts_sb[h // 2][:, half * 4:(half + 1) * 4, po:po + D],
                    yps[:, :, 0:D],
                    inv[:, half * 4:(half + 1) * 4].unsqueeze(2)
                    .broadcast_to((128, 4, D)))

            def transpose_pair(pair):
                rt = pstile(f"y4ps{pair}")
                y4ps = rt[:].bitcast(BF16)[:, 0:LQ]
                for ich in range(NI):
                    nc.tensor.transpose(y4ps[:, ich * 128:(ich + 1) * 128],
                                        yts_sb[pair][:, ich, :], idn_sb[:])
                y4 = y4p.tile([128, LQ], BF16, tag="y4", name=f"y4_{pair}")
                nc.scalar.copy(y4[:], y4ps)
                y4_sb.append(y4)

            # slot plan: remaining projections and conversions placed into
            # specific (h, jt) slots so their psum use and conv engine time
            # hide under the sim/exp/yt stream. vt emission must stay ahead
            # of the same head's yt units (group g needs vt 4g..4g+3) but
            # behind its Pool xn chunk so PE wait-queue parking is short.
            H0_VT = {1: [0], 2: [1], 3: [2], 4: [3], 5: [4], 6: [5],
                     8: [6], 9: [7], 10: [8], 11: [9], 12: [10], 13: [11],
                     14: [12]}

            VT_V = frozenset((6, 7, 10, 11, 14, 15))

            def slot(h, jt):
                if h == 0:
                    if jt in H0_VT:
                        for v in H0_VT[jt]:
                            make_vt(v, nc.vector if v in VT_V else nc.scalar)
                elif h == 1:
                    if jt == 0:
                        for v in (13, 14, 15):
                            make_vt(v, nc.vector if v in VT_V else nc.scalar)
                    elif jt == 3:
                        _late_dmas()
                    elif jt == 4:
                        make_q(1)
                    elif jt == 8:
                        make_k_half(1, 0, nc.vector)
                    elif jt == 12:
                        make_k_half(1, 1, nc.scalar)
                elif h == 2:
                    if jt == 6:
                        make_q(2)
                elif h == 3:
                    if jt == 4:
                        make_k_half(2, 0, nc.scalar)
                    elif jt == 10:
                        make_k_half(2, 1, nc.vector)
                elif h == 4:
                    if jt == 6:
                        make_q(3)
                elif h == 5:
                    if jt == 2:
                        _later_dmas()
                    elif jt == 4:
                        make_k_half(3, 0, nc.scalar)
                    elif jt == 10:
                        make_k_half(3, 1, nc.vector)
                elif h == 5 and jt == 2:
                    _later_dmas()

            def out_pair_partial(mt):
                ops = pstile(f"ops{mt}")
                out_ps.append(ops)
                for pair in range(3):
                    for n in range(LQ // 512):
                        nc.tensor.matmul(
                            ops[:, n * 512:(n + 1) * 512],
                            wo_sb[pair][:, mt * 128:(mt + 1) * 128],
                            y4_sb[pair][:, n * 512:(n + 1) * 512],
                            start=(pair == 0), stop=False)
                for n in range(LQ // 512):
                    # rank-1 bias accumulate: + b_out[c] * ones_row
                    nc.tensor.matmul(
                        ops[:, n * 512:(n + 1) * 512],
                        bo_row[0:1, mt * 128:(mt + 1) * 128],
                        ones_row[:], start=False, stop=False,
                        skip_group_check=True)

            # pair-3 transpose: ONE arena tile for both halves (ops0/ops1
            # pin two slots until the final stores, so a second rt tile
            # would serialize behind the single remaining slot).
            t3 = {}

            def transpose3_half(half):
                if half == 0:
                    t3["y4"] = y4p.tile([128, LQ], BF16, tag="y4",
                                        name="y4_3")
                    y4_sb.append(t3["y4"])
                    t3["rt"] = pstile("y4ps3")
                y4ps = t3["rt"][:].bitcast(BF16)[:, 0:LQ]
                hs = slice(half * 512, (half + 1) * 512)
                for i in range(4):
                    ich = half * 4 + i
                    nc.tensor.transpose(y4ps[:, ich * 128:(ich + 1) * 128],
                                        yts_sb[3][:, ich, :], idn_sb[:])
                nc.vector.tensor_copy(t3["y4"][:, hs], y4ps[:, hs])

            YT_UNITS = {7: (0, 0), 8: (0, 1), 11: (1, 0), 12: (1, 1),
                        15: (2, 0)}
            YT_SPILL = {0: (2, 1), 1: (3, 0), 2: (3, 1)}
            for h in range(H + 1):
                if h < H:
                    pt_sb[h] = ptp.tile([128, NJ, LQ], BF16, tag="pt",
                                        name=f"pt{h}")
                    if h % 2 == 0:
                        yts_sb[h // 2] = ytsp.tile([128, NI, 128], BF16,
                                                   tag="yts",
                                                   name=f"yts{h // 2}")
                for jt in range(NJ):
                    if h < H:
                        sim_chunk(h, jt)
                    slot(h, jt)
                    if h < H:
                        if jt == 6:
                            yt_alloc(h)
                        elif jt in YT_UNITS:
                            yt_unit(h, *YT_UNITS[jt])
                    if h > 0 and jt in YT_SPILL:
                        yt_unit(h - 1, *YT_SPILL[jt])
                    if h > 0 and jt in (4, 5) and h <= H:
                        # finishes deferred past the early exps so the DVE
                        # drains the sim ring before the recip+mul pair
                        yt_finish(h - 1, jt - 4)
                    if jt == 8 and h in (2, 4, 6):
                        transpose_pair(h // 2 - 1)
                    if h == H:
                        # out-proj partials first: they depend only on the
                        # long-ready pairs 0-2, so they must not queue
                        # behind the pair-3 transposes in the PE sequencer.
                        if jt == 1:
                            out_pair_partial(0)
                        elif jt == 2:
                            out_pair_partial(1)
                        elif jt == 5:
                            transpose3_half(0)
                        elif jt == 6:
                            transpose3_half(1)

            # ------- pipelined out-proj tail: per 256-col block, finish the
            # pair-3 accumulation, bias, sum-sq, ln/exp rsqrt, scale, store.
            # ss2 accumulators live in the yT psum banks (free by now);
            # block pairs alternate between the two so a block's start=True
            # bank reset never waits on the previous block's Ln read. The
            # bias already rode the out-proj psum, so staging to SBUF is a
            # plain copy; everything downstream reads SBUF (a PSUM-direct
            # tail serializes blocks on bank-level WARs).
            out_sb = outp.tile([128, 2, LQ], BF16, tag="osb")
            ss2ps = [ps_yt.tile([128, 512], F32, tag="yt",
                                name=f"ss2ps{i}") for i in range(2)]
            for nb in range(2):
                bs = slice(nb * 512, (nb + 1) * 512)
                for mt in range(2):
                    nc.tensor.matmul(
                        out_ps[mt][:, bs],
                        wo_sb[3][:, mt * 128:(mt + 1) * 128],
                        t3["y4"][:, bs],
                        start=False, stop=True, skip_group_check=True)
                nc.vector.tensor_copy(out_sb[:, 0, bs], out_ps[0][:, bs])
                nc.scalar.copy(out_sb[:, 1, bs], out_ps[1][:, bs])
                sq2b = sqp.tile([128, 2, 512], BF16, tag="sq2",
                                name=f"sq2_{nb}")
                nc.vector.tensor_mul(sq2b[:], out_sb[:, :, bs],
                                     out_sb[:, :, bs])
                for mt in range(2):
                    nc.tensor.matmul(ss2ps[nb][:], ones128[:],
                                     sq2b[:, mt, :],
                                     start=(mt == 0), stop=(mt == 1))
                s2ln = slnp.tile([128, 512], F32, tag="sln",
                                 name=f"s2ln{nb}")
                nc.scalar.activation(s2ln[:], ss2ps[nb][:], AF.Ln,
                                     bias=eps_t[:], scale=1.0 / C)
                s2bc = sbcp.tile([128, 512], F32, tag="sbc",
                                 name=f"s2bc{nb}")
                nc.scalar.activation(s2bc[:], s2ln[:], AF.Exp,
                                     bias=zero_t[:], scale=-0.5)
                fin = finp.tile([128, 2, 512], F32, tag="fin",
                                name=f"fin{nb}")
                for mt in range(2):
                    nc.vector.scalar_tensor_tensor(
                        fin[:, mt, :], out_sb[:, mt, bs], g2_sb[mt][:],
                        s2bc[:], op0=MUL, op1=MUL)
                for mt in range(2):
                    nc.sync.dma_start(out[mt * 128:(mt + 1) * 128, bs],
                                      fin[:, mt, :])


_NC = None


def _get_nc():
    global _NC
    if _NC is None:
        nc = bacc.Bacc("TRN2", target_bir_lowering=False, debug=False,
                       enable_asserts=False, num_devices=8)
        x_d = nc.dram_tensor("x", [C, L], BF16, kind="ExternalInput")
        wq_d = nc.dram_tensor("wqkvT", [C, 3 * HID], BF16, kind="ExternalInput")
        wo_d = nc.dram_tensor("woutT", [HID, C], BF16, kind="ExternalInput")
        b_d = nc.dram_tensor("bout", [1, C], BF16, kind="ExternalInput")
        g2_d = nc.dram_tensor("g2v", [C, 1], F32, kind="ExternalInput")
        idn_d = nc.dram_tensor("idn", [128, 128], BF16, kind="ExternalInput")
        out_d = nc.dram_tensor("out", [C, LQ], F32, kind="ExternalOutput")
        with tile.TileContext(nc) as tc:
            _body(tc, x_d.ap(), wq_d.ap(), wo_d.ap(), b_d.ap(), g2_d.ap(),
                  idn_d.ap(), out_d.ap())
        nc.compile()
        _NC = nc
    return _NC


def _in_maps(x, g1, w_qkv, w_out, b_out, g2):
    BFH = ml_dtypes.bfloat16
    w2 = (np.asarray(w_qkv, np.float32)
          * np.asarray(g1, np.float32).reshape(1, C))
    wqkvT = np.ascontiguousarray(w2.T).astype(BFH)
    woutT = np.ascontiguousarray(np.asarray(w_out, np.float32).T).astype(BFH)
    bo = np.asarray(b_out, np.float32).reshape(1, C).astype(BFH)
    g2v = np.asarray(g2, np.float32).reshape(C, 1)
    idn = np.eye(128, dtype=BFH)
    maps = []
    for core in range(8):
        b, half = divmod(core, 2)
        xb = np.asarray(x[b], np.float32)
        x_core = np.ascontiguousarray(np.concatenate(
            [xb[:, half * LQ:(half + 1) * LQ],
             xb[:, (1 - half) * LQ:(2 - half) * LQ]], axis=1)).astype(BFH)
        maps.append({"x": x_core, "wqkvT": wqkvT, "woutT": woutT,
                     "bout": bo, "g2v": g2v, "idn": idn})
    return maps


def _assemble(results):
    out = np.empty((B, C, L), np.float32)
    for core in range(8):
        b, half = divmod(core, 2)
        out[b][:, half * LQ:(half + 1) * LQ] = results[core]["out"]
    return out


def kernel(x, g1, w_qkv, w_out, b_out, g2, _trace=False, _tmpdir=None):
    res = run_bass_kernel_spmd(_get_nc(),
                               _in_maps(x, g1, w_qkv, w_out, b_out, g2),
                               core_ids=list(range(8)), trace=_trace,
                               tmpdir=_tmpdir)
    out = _assemble(res.results)
    if _trace:
        return out, res
    return out


# revision 92
# speedup vs baseline: 1.0452x; 1.0168x over previous
"""Trainium2 Bass kernel for the attention module (b=4, c=256, l=2048, h=8, d=64).

Sharding: 8 cores = 4 batches x 2 query-halves (no collectives). Each core
receives its batch's x with columns permuted so its own query half comes
first; it computes k/v for all 2048 key positions and the attention output
for its 1024 queries, then the output projection + final rms-norm.

Device algorithm per core (all matmul data bf16, psum fp32):
  ss     = ones128.T @ bf16(x*x)         (PE bcast-reduce: rows arrive
                                          pre-broadcast; plus 1-col reduces
                                          for a transposed copy s_t)
  s      = exp(-0.5*ln(ss/256+eps))      (ACT Ln+Exp rsqrt: one
                                          natural_log_exp ACT table covers
                                          every function in the kernel, so
                                          exactly one table load, ever)
  q      = bf16((wq.T @ x) * s_bcA)      (norm scale folded at the psum->
                                          sbuf conversion; g1 on host)
  k      = bf16(wk.T @ x)                (UNSCALED: the key-side s rides
                                          the exp as a per-partition scale)
  vT     = bf16((x_slice.T @ wv) * s_t)  (per-partition s_t scale, either
                                          engine; fused ones col for den)
  per head:  simT[j,i] = k_h.T @ q_h     (psum fp32, logits*8*s_j)
             pt = exp(simT*s_t/8)  bf16  (ACT true exp with scale=s_t8 AP /
                                          DVE Schraudolph bitcast-exp with
                                          scalar=s_tA AP, split per table)
             ytps[i, d|den] += pt.T @ vT (jt-major groups, lag ~5 jt behind
                                          the same head's sim/exp stream)
  y_h    = ytps[:, 0:64] * (1/den)       (DVE recip + stride-0 bcast mul)
  y4     = PE transpose(yts, identity)   (bf16, per head-pair)
  out    = woT.T @ y4 (+ b_out as a K=1 rank-1 matmul into the same psum)
  result = out * g2 * exp(-0.5*ln(ss2/256+eps))   (bcast-reduce again)

Scheduling: 512-col-chunked input norm emitted under tc.high_priority so
the list scheduler runs the s chain ahead of the projection matmuls; PE
pstate-warmup dummy matmuls during the initial DMA wait; one rotating
3x[128,1024] psum arena for projections, sim chunks, pair transposes and
the out-proj; yT units interleave into their own head's stream with only
the last jt-group spilling into the next head; per-head exp engine tables
(EXP_ENG) balance ACT/DVE just under the PE roofline, with h7's last two
tiles split across both engines to shorten the tail; q/k projections for
later heads spread across h1-h5 slots; 2x512-col pipelined output tail
(pair-3 out-proj, copy, sum-sq, ln/exp, scale, store). x/wq ride the SP
HWDGE queue (x moves both c-tiles per transfer via an AP rearrange);
idn/wo/bo/g2 ride the gpsimd SWDGE queue late.
"""

import sys

import numpy as np

if "/opt/trn_rl_repo" not in sys.path:
    sys.path.insert(0, "/opt/trn_rl_repo")

import ml_dtypes  # noqa: E402

import concourse.bass as bass  # noqa: E402
import concourse.tile as tile  # noqa: E402
from concourse import bacc, mybir  # noqa: E402
from concourse.bass_utils import run_bass_kernel_spmd  # noqa: E402

F32 = mybir.dt.float32
F32R = mybir.dt.float32r
BF16 = mybir.dt.bfloat16
U16 = mybir.dt.uint16
AF = mybir.ActivationFunctionType
MUL = mybir.AluOpType.mult
ADD = mybir.AluOpType.add

B, C, L = 4, 256, 2048
H, D = 8, 64
HID = H * D
LQ = L // 2      # queries per core
NCT = C // 128   # 2 c-tiles
NJ = L // 128    # 16 j-tiles
NI = LQ // 128   # 8 i-chunks
EPS_B = 1e-26

LOG2E = 1.4426950408889634
# Schraudolph bf16: bits = round(logit*128*log2e + 127*128 - 5.6); logit = sim/8
SCH_A = 128.0 * LOG2E / 8.0
SCH_B = 127.0 * 128.0 - 5.6

# exp engine split per (h, jt): 'A' ACT true exp, 'V' DVE Schraudolph.
# (PSUM cannot be DMA'd and Pool cannot read PSUM, so these two engines
# carry all 128 exp tiles; the ratio balances their total load just under
# the PE roofline.) Alternation keeps the 3-deep sim psum ring draining
# on both engines at once.
_ROW8A = ['A', 'V', 'A', 'V', 'A', 'V', 'A', 'V',
          'A', 'V', 'A', 'V', 'A', 'V', 'A', 'V']
_ROW9A = ['A', 'V', 'A', 'V', 'A', 'A', 'V', 'A',
          'V', 'A', 'A', 'V', 'A', 'V', 'A', 'A']
# h0's DVE is busy with the s-folded q/k conversions until ~jt6, so its
# early exps go to ACT.
_ROWH0 = ['A', 'A', 'A', 'A', 'A', 'A', 'A', 'V',
          'A', 'V', 'A', 'V', 'A', 'V', 'A', 'V']
_ROWH7 = ['A', 'V', 'A', 'V', 'A', 'A', 'V', 'A',
          'V', 'A', 'A', 'V', 'A', 'V', 'S', 'S']
EXP_ENG = ([list(_ROWH0)]
           + [list(_ROW9A) if h % 2 else list(_ROW8A)
              for h in range(1, H - 1)]
           + [list(_ROWH7)])


def _body(tc, x, wq, wo, bo, g2, idn, out):
    nc = tc.nc
    from contextlib import ExitStack
    with ExitStack() as ctx:
        ctx.enter_context(nc.allow_low_precision(
            reason="bf16 data path by design"))
        const = ctx.enter_context(tc.tile_pool(name="const", bufs=1))
        bigx = ctx.enter_context(tc.tile_pool(name="bigx", bufs=1))
        sqp = ctx.enter_context(tc.tile_pool(name="sq", bufs=2))
        slnp = ctx.enter_context(tc.tile_pool(name="sln", bufs=2))
        sbnp = ctx.enter_context(tc.tile_pool(name="sbn", bufs=1))
        sbcp = ctx.enter_context(tc.tile_pool(name="sbc", bufs=4))
        qp = ctx.enter_context(tc.tile_pool(name="q", bufs=4))
        kp = ctx.enter_context(tc.tile_pool(name="k", bufs=4))
        vtp = ctx.enter_context(tc.tile_pool(name="vt", bufs=16))
        ptp = ctx.enter_context(tc.tile_pool(name="pt", bufs=2))
        invp = ctx.enter_context(tc.tile_pool(name="inv", bufs=2))
        ytsp = ctx.enter_context(tc.tile_pool(name="yts", bufs=2))
        y4p = ctx.enter_context(tc.tile_pool(name="y4", bufs=4))
        outp = ctx.enter_context(tc.tile_pool(name="outp", bufs=1))
        finp = ctx.enter_context(tc.tile_pool(name="fin", bufs=2))

        # ---------------- constants ----------------
        # Preload the one ACT table that covers every function this kernel
        # uses (Ln, Exp, Copy, Identity). Bacc's auto-inserter is greedy
        # per-function and would thrash 13 loads between the single-function
        # tables; with this explicit load it sees every activation covered.
        from concourse.hw_specs import get_activation_tables
        tabs = list(get_activation_tables(nc.m.arch))
        joint_id = tabs.index("natural_log_exp_and_others")
        nc.scalar.add_instruction(mybir.InstLoadActFuncSet(
            name=nc.get_next_instruction_name(),
            act_func_set_id=joint_id, engine=mybir.EngineType.Activation,
            ins=[], outs=[]))

        dmyr = const.tile([128, 512], BF16, tag="dmyr")
        nc.gpsimd.memset(dmyr[:], 0.0)
        ones128 = const.tile([128, 128], BF16, tag="ones128")
        nc.vector.memset(ones128[:], 1.0)
        ones_row = const.tile([1, 512], BF16, tag="ones_row")
        nc.vector.memset(ones_row[:], 1.0)
        stage = const.tile([128, 8], F32, tag="stage")
        nc.gpsimd.memset(stage[:], 1.0)
        eps_t = const.tile([128, 1], F32, tag="eps")
        nc.gpsimd.memset(eps_t[:], EPS_B)
        zero_t = const.tile([128, 1], F32, tag="zero")
        nc.gpsimd.memset(zero_t[:], 0.0)

        # ---------------- input DMAs ----------------
        # x as four 512-col chunk DMAs, each moving BOTH c-tiles in one
        # transfer (256 dram rows -> [128, 2, 512] sbuf) so only 4 HWDGE
        # setups serialize. wq rides the DVE queue so the ACT sequencer
        # stays free for the norm-chain Ln/Exp.
        x_sb = bigx.tile([128, NCT, L], BF16, tag="x")
        wq_sb = []
        for ct in range(NCT):
            wq_sb.append(const.tile([128, 3 * HID], BF16, tag=f"wq{ct}",
                                    name=f"wq{ct}"))
        nc.sync.dma_start(
            x_sb[:, :, 0:1024],
            x[:, 0:1024].rearrange("(a p) c -> p a c", a=NCT))
        for ct in range(NCT):
            nc.sync.dma_start(wq_sb[ct][:, 0:HID],
                              wq[ct * 128:(ct + 1) * 128, 0:HID])
        nc.sync.dma_start(
            x_sb[:, :, 1024:2048],
            x[:, 1024:2048].rearrange("(a p) c -> p a c", a=NCT))
        for part in range(1, 3):
            for ct in range(NCT):
                nc.sync.dma_start(
                    wq_sb[ct][:, part * HID:(part + 1) * HID],
                    wq[ct * 128:(ct + 1) * 128, part * HID:(part + 1) * HID])

        # idn/wo/bo/g2 are needed only late; ride the gpsimd SWDGE queue.
        idn_sb = const.tile([128, 128], BF16, tag="idn")
        wo_sb = [const.tile([128, C], BF16, tag=f"wo{kt}", name=f"wo{kt}")
                 for kt in range(4)]
        bo_row = const.tile([1, C], BF16, tag="bo_row")
        g2_sb = [const.tile([128, 1], F32, tag=f"g2{mt}", name=f"g2{mt}")
                 for mt in range(2)]

        def _late_dmas():
            nc.gpsimd.dma_start(idn_sb[:], idn)

        def _later_dmas():
            for kt in range(4):
                nc.gpsimd.dma_start(wo_sb[kt][:],
                                    wo[kt * 128:(kt + 1) * 128, :])
            nc.gpsimd.dma_start(bo_row[:], bo)
            for mt in range(2):
                nc.gpsimd.dma_start(g2_sb[mt][:],
                                    g2[mt * 128:(mt + 1) * 128, :])

        q_sb, k_sb = [None] * 4, [None] * 4
        vt_sb = [None] * NJ
        y4_sb = []

        with tc.tile_pool(name="ps", bufs=3, space="PSUM") as psp, \
                tc.tile_pool(name="ps_yt", bufs=2, space="PSUM") as ps_yt:

            def pstile(name):
                return psp.tile([128, 1024], F32, tag="sim", name=name)

            # --- PE pstate warmup: zero matmuls chained on one arena tile
            # while the first x chunks stream in, so the real front matmuls
            # run at full clock. Output is never read. ---
            dmy_ps = pstile("dmy")
            for i in range(7):
                nc.tensor.matmul(dmy_ps[:8, 0:512], ones128[:, 0:8],
                                 dmyr[:], start=True, stop=True)

            # --- chunked input rms-norm: ss (PE bcast-reduce, rows arrive
            # pre-broadcast) -> ln (ACT) -> exp(-.5) (ACT) = rsqrt -> s_bc
            # (row-broadcast, for the per-column q/k scale-folding). Own
            # half (A) in two 512 chunks for a lean chain; far half (B) at
            # 1024 wide. A second, transposed copy s_t [128, NJ] comes from
            # 1-column reduces (out[p, 0] = ss[jt*128+p]) + a tiny ln/exp;
            # it feeds the vT conversions as a per-PARTITION scale, so vT
            # projects from raw x and xn never exists.
            hp = ctx.enter_context(tc.high_priority())
            s_bcA = sbnp.tile([128, 1024], F32, tag="sbcA", name="s_bcA")
            sq8a = sbnp.tile([128, NCT, 1024], BF16, tag="sq8", name="sq8a")
            for c in range(2):
                cs = slice(c * 512, (c + 1) * 512)
                nc.vector.tensor_mul(sq8a[:, :, cs], x_sb[:, :, cs],
                                     x_sb[:, :, cs])
                ssps = ps_yt.tile([128, 512], F32, tag="yt", name=f"ss{c}")
                for ct in range(NCT):
                    nc.tensor.matmul(ssps[:], ones128[:],
                                     sq8a[:, ct, cs],
                                     start=(ct == 0), stop=(ct == NCT - 1))
                s_ln = slnp.tile([128, 512], F32, tag="sln", name=f"sln{c}")
                nc.scalar.activation(s_ln[:], ssps[:], AF.Ln,
                                     bias=eps_t[:], scale=1.0 / C)
                nc.scalar.activation(s_bcA[:, cs], s_ln[:], AF.Exp,
                                     bias=zero_t[:], scale=-0.5)
            sq8b = sbnp.tile([128, NCT, 1024], BF16, tag="sq8b", name="sq8b")
            nc.vector.tensor_mul(sq8b[:], x_sb[:, :, 1024:2048],
                                 x_sb[:, :, 1024:2048])
            # transposed s: one 1-column reduce per key tile. One shared
            # start (pending-zero covers the whole bank region; each
            # column's first write overwrites, the second accumulates).
            sstp = ps_yt.tile([128, NJ], F32, tag="yt", name="sstp")
            for jt in range(NJ):
                sq8v = (sq8a if jt < 8 else sq8b)
                jo = (jt % 8) * 128
                for ct in range(NCT):
                    nc.tensor.matmul(
                        sstp[:, jt:jt + 1],
                        sq8v[:, ct, jo:jo + 128],
                        ones128[:, 0:1],
                        start=(jt == 0 and ct == 0),
                        stop=(jt == NJ - 1 and ct == NCT - 1),
                        skip_group_check=True)
            st_ln = slnp.tile([128, NJ], F32, tag="stln", name="st_ln")
            nc.scalar.activation(st_ln[:], sstp[:], AF.Ln,
                                 bias=eps_t[:], scale=1.0 / C)
            s_t = sbnp.tile([128, NJ], F32, tag="st", name="s_t")
            nc.scalar.activation(s_t[:], st_ln[:], AF.Exp,
                                 bias=zero_t[:], scale=-0.5)
            # key-side norm scale folded into the exp: per-partition scale
            # APs (sim partitions ARE keys), pre-multiplied by the logit
            # scales of the two exp flavors.
            s_t8 = sbnp.tile([128, NJ], F32, tag="st8", name="s_t8")
            nc.vector.tensor_scalar_mul(s_t8[:], s_t[:], 0.125)
            s_tA = sbnp.tile([128, NJ], F32, tag="stA", name="s_tA")
            nc.vector.tensor_scalar_mul(s_tA[:], s_t[:], SCH_A)
            ctx.pop_all().close() if False else None

            # --- projection helpers (psum from the shared arena) ---
            # All projections run on RAW x; the norm scale s is applied at
            # the psum->sbuf conversion (mathematically identical since s
            # is per-column). For q/k that's a per-column mul with the
            # row-broadcast s (DVE tensor_tensor); for vT the column index
            # is the PARTITION, so the transposed s_t rides a per-partition
            # scale and either engine can convert.
            def make_q(mt, split=False):
                ps = pstile(f"qps{mt}")
                for n in range(2):
                    for ct in range(NCT):
                        nc.tensor.matmul(
                            ps[:, n * 512:(n + 1) * 512],
                            wq_sb[ct][:, mt * 128:(mt + 1) * 128],
                            x_sb[:, ct, n * 512:(n + 1) * 512],
                            start=(ct == 0), stop=(ct == NCT - 1))
                t = qp.tile([128, LQ], BF16, tag="q", name=f"qsb{mt}")
                if split:
                    for c in range(2):
                        cs = slice(c * 512, (c + 1) * 512)
                        nc.vector.tensor_mul(t[:, cs], ps[:, cs],
                                             s_bcA[:, cs])
                else:
                    nc.vector.tensor_mul(t[:], ps[:, :], s_bcA[:])
                q_sb[mt] = t

            def make_k_half(mt, half, eng=None):
                if half == 0:
                    k_sb[mt] = kp.tile([128, L], BF16, tag="k",
                                       name=f"ksb{mt}")
                t = k_sb[mt]
                ps = pstile(f"kps{mt}_{half}")
                for n in range(2):
                    for ct in range(NCT):
                        nc.tensor.matmul(
                            ps[:, n * 512:(n + 1) * 512],
                            wq_sb[ct][:, HID + mt * 128:HID + (mt + 1) * 128],
                            x_sb[:, ct,
                                 half * 1024 + n * 512:
                                 half * 1024 + (n + 1) * 512],
                            start=(ct == 0), stop=(ct == NCT - 1))
                if eng is nc.scalar:
                    eng.copy(t[:, half * LQ:(half + 1) * LQ], ps[:, :])
                else:
                    nc.vector.tensor_copy(t[:, half * LQ:(half + 1) * LQ],
                                          ps[:, :])

            def make_vt(jt, eng):
                ps = pstile(f"vps{jt}")
                for ct in range(NCT):
                    nc.tensor.matmul(ps[:, 0:512],
                                     x_sb[:, ct, jt * 128:(jt + 1) * 128],
                                     wq_sb[ct][:, 2 * HID:3 * HID],
                                     start=(ct == 0), stop=(ct == NCT - 1))
                t = vtp.tile([128, H, D + 1], BF16, tag="vt", name=f"vt{jt}")
                src = ps[:, 0:512].rearrange("p (h e) -> p h e", e=D)
                if eng is nc.scalar:
                    eng.activation(t[:, :, 0:D], src, AF.Identity,
                                   bias=zero_t[:], scale=s_t[:, jt:jt + 1])
                else:
                    eng.tensor_scalar_mul(t[:, :, 0:D], src,
                                          s_t[:, jt:jt + 1])
                nc.gpsimd.tensor_copy(
                    t[:, :, D:D + 1].rearrange("p h o -> p (h o)"),
                    stage[:, 0:H])
                vt_sb[jt] = t

            # Minimal prefix for sim h0: q mt0, k mt0 both halves; they
            # depend only on x + wq + the s chain. vT tiles are emitted
            # inside h0's slots.
            make_q(0, split=True)
            make_k_half(0, 0, nc.scalar)
            make_k_half(0, 1, nc.scalar)

            # --- attention: head h sims + exp stream while head h-1's yT
            # accumulates from its fully-materialized pt tile; remaining
            # projections interleave into h0-h3's slots. ---
            pt_sb = [None] * H
            yts_sb = [None] * 4
            out_ps = []
            state = {}

            def sim_chunk(h, jt):
                mt, po = h // 2, (h % 2) * D
                sps = pstile(f"sps{h}_{jt}")
                for n in range(LQ // 512):
                    nc.tensor.matmul(
                        sps[:, n * 512:(n + 1) * 512],
                        k_sb[mt][po:po + D, jt * 128:(jt + 1) * 128],
                        q_sb[mt][po:po + D, n * 512:(n + 1) * 512],
                        start=True, stop=True)
                dst = pt_sb[h][:, jt, :]
                code = EXP_ENG[h][jt]
                stA = s_tA[:, jt:jt + 1]
                st8 = s_t8[:, jt:jt + 1]
                if code == 'V':
                    nc.vector.tensor_scalar(
                        dst.bitcast(U16), sps[:, :], stA, SCH_B,
                        op0=MUL, op1=ADD)
                elif code == 'S':
                    nc.scalar.activation(dst[:, 0:512], sps[:, 0:512],
                                         AF.Exp, bias=zero_t[:], scale=st8)
                    nc.vector.tensor_scalar(
                        dst[:, 512:1024].bitcast(U16), sps[:, 512:1024],
                        stA, SCH_B, op0=MUL, op1=ADD)
                else:
                    nc.scalar.activation(dst, sps[:, :], AF.Exp,
                                         bias=zero_t[:], scale=st8)

            # yT accumulation runs lag-~5-jt behind the SAME head's sim/exp
            # stream (jt-major groups make this legal), so only the last
            # jt-group spills past the head's final sim -- the old
            # one-full-head lag put the whole last yT serially in the tail.
            def yt_alloc(h):
                y0 = ps_yt.tile([128, 4, D + 1], F32, tag="yt",
                                name=f"yt{h}_0")
                y1 = ps_yt.tile([128, 4, D + 1], F32, tag="yt",
                                name=f"yt{h}_1")
                state[h] = {"yps": [y0, y1]}

            def yt_unit(h, g, half):
                """16 matmuls: jt-group 4g..4g+3 x 4 query-chunks into the
                `half` accumulator."""
                yps, pt = state[h]["yps"][half], pt_sb[h]
                for ic in range(4):
                    for jt in range(4 * g, 4 * g + 4):
                        nc.tensor.matmul(
                            yps[:, ic, :],
                            pt[:, jt, (half * 4 + ic) * 128:
                               (half * 4 + ic + 1) * 128],
                            vt_sb[jt][:, h, :],
                            start=(g == 0 and ic == 0 and jt == 0),
                            stop=(jt == NJ - 1),
                            skip_group_check=True)

            def yt_finish(h, half):
                yps = state[h]["yps"][half]
                po = (h % 2) * D
                if half == 0:
                    inv = invp.tile([128, 8], F32, tag="inv",
                                    name=f"inv{h}")
                    state[h]["inv"] = inv
                else:
                    inv = state[h]["inv"]
                nc.vector.reciprocal(
                    inv[:, half * 4:(half + 1) * 4],
                    yps[:, :, D:D + 1].rearrange("p a b -> p (a b)"))
                nc.vector.tensor_mul(
                    yts_sb[h // 2][:, half * 4:(half + 1) * 4, po:po + D],
                    yps[:, :, 0:D],
                    inv[:, half * 4:(half + 1) * 4].unsqueeze(2)
                    .broadcast_to((128, 4, D)))

            def transpose_pair(pair):
                rt = pstile(f"y4ps{pair}")
                y4ps = rt[:].bitcast(BF16)[:, 0:LQ]
                for ich in range(NI):
                    nc.tensor.transpose(y4ps[:, ich * 128:(ich + 1) * 128],
                                        yts_sb[pair][:, ich, :], idn_sb[:])
                y4 = y4p.tile([128, LQ], BF16, tag="y4", name=f"y4_{pair}")
                nc.scalar.copy(y4[:], y4ps)
                y4_sb.append(y4)

            # slot plan: remaining projections and conversions placed into
            # specific (h, jt) slots so their psum use and conv engine time
            # hide under the sim/exp/yt stream. vt emission must stay ahead
            # of the same head's yt units (group g needs vt 4g..4g+3) but
            # behind its Pool xn chunk so PE wait-queue parking is short.
            H0_VT = {1: [0], 2: [1], 3: [2], 4: [3], 5: [4], 6: [5],
                     8: [6], 9: [7], 10: [8], 11: [9], 12: [10], 13: [11],
                     14: [12]}

            VT_V = frozenset((0, 1, 2, 3, 4, 5, 9, 11, 13, 15))

            def slot(h, jt):
                if h == 0:
                    if jt in H0_VT:
                        for v in H0_VT[jt]:
                            make_vt(v, nc.vector if v in VT_V else nc.scalar)
                elif h == 1:
                    if jt == 0:
                        for v in (13, 14, 15):
                            make_vt(v, nc.vector if v in VT_V else nc.scalar)
                    elif jt == 3:
                        _late_dmas()
                    elif jt == 4:
                        make_q(1)
                    elif jt == 8:
                        make_k_half(1, 0, nc.vector)
                    elif jt == 12:
                        make_k_half(1, 1, nc.scalar)
                elif h == 2:
                    if jt == 6:
                        make_q(2)
                elif h == 3:
                    if jt == 4:
                        make_k_half(2, 0, nc.scalar)
                    elif jt == 10:
                        make_k_half(2, 1, nc.vector)
                elif h == 4:
                    if jt == 6:
                        make_q(3)
                elif h == 5:
                    if jt == 2:
                        _later_dmas()
                    elif jt == 4:
                        make_k_half(3, 0, nc.scalar)
                    elif jt == 10:
                        make_k_half(3, 1, nc.vector)
                elif h == 5 and jt == 2:
                    _later_dmas()

            def out_pair_partial(mt):
                ops = pstile(f"ops{mt}")
                out_ps.append(ops)
                for pair in range(3):
                    for n in range(LQ // 512):
                        nc.tensor.matmul(
                            ops[:, n * 512:(n + 1) * 512],
                            wo_sb[pair][:, mt * 128:(mt + 1) * 128],
                            y4_sb[pair][:, n * 512:(n + 1) * 512],
                            start=(pair == 0), stop=False)
                for n in range(LQ // 512):
                    # rank-1 bias accumulate: + b_out[c] * ones_row
                    nc.tensor.matmul(
                        ops[:, n * 512:(n + 1) * 512],
                        bo_row[0:1, mt * 128:(mt + 1) * 128],
                        ones_row[:], start=False, stop=False,
                        skip_group_check=True)

            # pair-3 transpose: ONE arena tile for both halves (ops0/ops1
            # pin two slots until the final stores, so a second rt tile
            # would serialize behind the single remaining slot).
            t3 = {}

            def transpose3_half(half):
                if half == 0:
                    t3["y4"] = y4p.tile([128, LQ], BF16, tag="y4",
                                        name="y4_3")
                    y4_sb.append(t3["y4"])
                    t3["rt"] = pstile("y4ps3")
                y4ps = t3["rt"][:].bitcast(BF16)[:, 0:LQ]
                hs = slice(half * 512, (half + 1) * 512)
                for i in range(4):
                    ich = half * 4 + i
                    nc.tensor.transpose(y4ps[:, ich * 128:(ich + 1) * 128],
                                        yts_sb[3][:, ich, :], idn_sb[:])
                nc.vector.tensor_copy(t3["y4"][:, hs], y4ps[:, hs])

            YT_UNITS = {7: (0, 0), 8: (0, 1), 11: (1, 0), 12: (1, 1),
                        15: (2, 0)}
            YT_SPILL = {0: (2, 1), 1: (3, 0), 2: (3, 1)}
            for h in range(H + 1):
                if h < H:
                    pt_sb[h] = ptp.tile([128, NJ, LQ], BF16, tag="pt",
                                        name=f"pt{h}")
                    if h % 2 == 0:
                        yts_sb[h // 2] = ytsp.tile([128, NI, 128], BF16,
                                                   tag="yts",
                                                   name=f"yts{h // 2}")
                for jt in range(NJ):
                    if h < H:
                        sim_chunk(h, jt)
                    slot(h, jt)
                    if h < H:
                        if jt == 6:
                            yt_alloc(h)
                        elif jt in YT_UNITS:
                            yt_unit(h, *YT_UNITS[jt])
                    if h > 0 and jt in YT_SPILL:
                        yt_unit(h - 1, *YT_SPILL[jt])
                    if h > 0 and jt in (4, 5) and h <= H:
                        # finishes deferred past the early exps so the DVE
                        # drains the sim ring before the recip+mul pair
                        yt_finish(h - 1, jt - 4)
                    if jt == 8 and h in (2, 4, 6):
                        transpose_pair(h // 2 - 1)
                    if h == H:
                        # out-proj partials first: they depend only on the
                        # long-ready pairs 0-2, so they must not queue
                        # behind the pair-3 transposes in the PE sequencer.
                        if jt == 1:
                            out_pair_partial(0)
                        elif jt == 2:
                            out_pair_partial(1)
                        elif jt == 5:
                            transpose3_half(0)
                        elif jt == 6:
                            transpose3_half(1)

            # ------- pipelined out-proj tail: per 256-col block, finish the
            # pair-3 accumulation, bias, sum-sq, ln/exp rsqrt, scale, store.
            # ss2 accumulators live in the yT psum banks (free by now);
            # block pairs alternate between the two so a block's start=True
            # bank reset never waits on the previous block's Ln read. The
            # bias already rode the out-proj psum, so staging to SBUF is a
            # plain copy; everything downstream reads SBUF (a PSUM-direct
            # tail serializes blocks on bank-level WARs).
            out_sb = outp.tile([128, 2, LQ], BF16, tag="osb")
            ss2ps = [ps_yt.tile([128, 512], F32, tag="yt",
                                name=f"ss2ps{i}") for i in range(2)]
            for nb in range(2):
                bs = slice(nb * 512, (nb + 1) * 512)
                for mt in range(2):
                    nc.tensor.matmul(
                        out_ps[mt][:, bs],
                        wo_sb[3][:, mt * 128:(mt + 1) * 128],
                        t3["y4"][:, bs],
                        start=False, stop=True, skip_group_check=True)
                nc.vector.tensor_copy(out_sb[:, 0, bs], out_ps[0][:, bs])
                nc.scalar.copy(out_sb[:, 1, bs], out_ps[1][:, bs])
                sq2b = sqp.tile([128, 2, 512], BF16, tag="sq2",
                                name=f"sq2_{nb}")
                nc.vector.tensor_mul(sq2b[:], out_sb[:, :, bs],
                                     out_sb[:, :, bs])
                for mt in range(2):
                    nc.tensor.matmul(ss2ps[nb][:], ones128[:],
                                     sq2b[:, mt, :],
                                     start=(mt == 0), stop=(mt == 1))
                s2ln = slnp.tile([128, 512], F32, tag="sln",
                                 name=f"s2ln{nb}")
                nc.scalar.activation(s2ln[:], ss2ps[nb][:], AF.Ln,
                                     bias=eps_t[:], scale=1.0 / C)
                s2bc = sbcp.tile([128, 512], F32, tag="sbc",
                                 name=f"s2bc{nb}")
                nc.scalar.activation(s2bc[:], s2ln[:], AF.Exp,
                                     bias=zero_t[:], scale=-0.5)
                fin = finp.tile([128, 2, 512], F32, tag="fin",
                                name=f"fin{nb}")
                for mt in range(2):
                    nc.vector.scalar_tensor_tensor(
                        fin[:, mt, :], out_sb[:, mt, bs], g2_sb[mt][:],
                        s2bc[:], op0=MUL, op1=MUL)
                for mt in range(2):
                    nc.sync.dma_start(out[mt * 128:(mt + 1) * 128, bs],
                                      fin[:, mt, :])


_NC = None


def _get_nc():
    global _NC
    if _NC is None:
        nc = bacc.Bacc("TRN2", target_bir_lowering=False, debug=False,
                       enable_asserts=False, num_devices=8)
        x_d = nc.dram_tensor("x", [C, L], BF16, kind="ExternalInput")
        wq_d = nc.dram_tensor("wqkvT", [C, 3 * HID], BF16, kind="ExternalInput")
        wo_d = nc.dram_tensor("woutT", [HID, C], BF16, kind="ExternalInput")
        b_d = nc.dram_tensor("bout", [1, C], BF16, kind="ExternalInput")
        g2_d = nc.dram_tensor("g2v", [C, 1], F32, kind="ExternalInput")
        idn_d = nc.dram_tensor("idn", [128, 128], BF16, kind="ExternalInput")
        out_d = nc.dram_tensor("out", [C, LQ], F32, kind="ExternalOutput")
        with tile.TileContext(nc) as tc:
            _body(tc, x_d.ap(), wq_d.ap(), wo_d.ap(), b_d.ap(), g2_d.ap(),
                  idn_d.ap(), out_d.ap())
        nc.compile()
        _NC = nc
    return _NC


def _in_maps(x, g1, w_qkv, w_out, b_out, g2):
    BFH = ml_dtypes.bfloat16
    w2 = (np.asarray(w_qkv, np.float32)
          * np.asarray(g1, np.float32).reshape(1, C))
    wqkvT = np.ascontiguousarray(w2.T).astype(BFH)
    woutT = np.ascontiguousarray(np.asarray(w_out, np.float32).T).astype(BFH)
    bo = np.asarray(b_out, np.float32).reshape(1, C).astype(BFH)
    g2v = np.asarray(g2, np.float32).reshape(C, 1)
    idn = np.eye(128, dtype=BFH)
    maps = []
    for core in range(8):
        b, half = divmod(core, 2)
        xb = np.asarray(x[b], np.float32)
        x_core = np.ascontiguousarray(np.concatenate(
            [xb[:, half * LQ:(half + 1) * LQ],
             xb[:, (1 - half) * LQ:(2 - half) * LQ]], axis=1)).astype(BFH)
        maps.append({"x": x_core, "wqkvT": wqkvT, "woutT": woutT,
                     "bout": bo, "g2v": g2v, "idn": idn})
    return maps


def _assemble(results):
    out = np.empty((B, C, L), np.float32)
    for core in range(8):
        b, half = divmod(core, 2)
        out[b][:, half * LQ:(half + 1) * LQ] = results[core]["out"]
    return out


def kernel(x, g1, w_qkv, w_out, b_out, g2, _trace=False, _tmpdir=None):
    res = run_bass_kernel_spmd(_get_nc(),
                               _in_maps(x, g1, w_qkv, w_out, b_out, g2),
                               core_ids=list(range(8)), trace=_trace,
                               tmpdir=_tmpdir)
    out = _assemble(res.results)
    if _trace:
        return out, res
    return out


# revision 101
# speedup vs baseline: 1.0472x; 1.0019x over previous
"""Trainium2 Bass kernel for the attention module (b=4, c=256, l=2048, h=8, d=64).

Sharding: 8 cores = 4 batches x 2 query-halves (no collectives). Each core
receives its batch's x with columns permuted so its own query half comes
first; it computes k/v for all 2048 key positions and the attention output
for its 1024 queries, then the output projection + final rms-norm.

Device algorithm per core (all matmul data bf16, psum fp32):
  ss     = ones128.T @ bf16(x*x)         (PE bcast-reduce: rows arrive
                                          pre-broadcast; plus 1-col reduces
                                          for a transposed copy s_t)
  s      = exp(-0.5*ln(ss/256+eps))      (ACT Ln+Exp rsqrt: one
                                          natural_log_exp ACT table covers
                                          every function in the kernel, so
                                          exactly one table load, ever)
  q      = bf16((wq.T @ x) * s_bcA)      (norm scale folded at the psum->
                                          sbuf conversion; g1 on host)
  k      = bf16(wk.T @ x)                (UNSCALED: the key-side s rides
                                          the exp as a per-partition scale)
  vT     = bf16((x_slice.T @ wv) * s_t)  (per-partition s_t scale, either
                                          engine; fused ones col for den)
  per head:  simT[j,i] = k_h.T @ q_h     (psum fp32, logits*8*s_j)
             pt = exp(simT*s_t/8)  bf16  (ACT true exp with scale=s_t8 AP /
                                          DVE Schraudolph bitcast-exp with
                                          scalar=s_tA AP, split per table)
             ytps[i, d|den] += pt.T @ vT (jt-major groups, lag ~5 jt behind
                                          the same head's sim/exp stream)
  y_h    = ytps[:, 0:64] * (1/den)       (DVE recip + stride-0 bcast mul)
  y4     = PE transpose(yts, identity)   (bf16, per head-pair)
  out    = woT.T @ y4 (+ b_out as a K=1 rank-1 matmul into the same psum)
  result = out * g2 * exp(-0.5*ln(ss2/256+eps))   (bcast-reduce again)

Scheduling: 512-col-chunked input norm emitted under tc.high_priority so
the list scheduler runs the s chain ahead of the projection matmuls; PE
pstate-warmup dummy matmuls during the initial DMA wait; one rotating
3x[128,1024] psum arena for projections, sim chunks, pair transposes and
the out-proj; yT units interleave into their own head's stream with only
the last jt-group spilling into the next head; per-head exp engine tables
(EXP_ENG) balance ACT/DVE just under the PE roofline, with h7's last two
tiles split across both engines to shorten the tail; q/k projections for
later heads spread across h1-h5 slots; 2x512-col pipelined output tail
(pair-3 out-proj, copy, sum-sq, ln/exp, scale, store). x/wq ride the SP
HWDGE queue (x moves both c-tiles per transfer via an AP rearrange);
idn/wo/bo/g2 ride the gpsimd SWDGE queue late.
"""

import sys

import numpy as np

if "/opt/trn_rl_repo" not in sys.path:
    sys.path.insert(0, "/opt/trn_rl_repo")

import ml_dtypes  # noqa: E402

import concourse.bass as bass  # noqa: E402
import concourse.tile as tile  # noqa: E402
from concourse import bacc, mybir  # noqa: E402
from concourse.bass_utils import run_bass_kernel_spmd  # noqa: E402

F32 = mybir.dt.float32
F32R = mybir.dt.float32r
BF16 = mybir.dt.bfloat16
U16 = mybir.dt.uint16
AF = mybir.ActivationFunctionType
MUL = mybir.AluOpType.mult
ADD = mybir.AluOpType.add

B, C, L = 4, 256, 2048
H, D = 8, 64
HID = H * D
LQ = L // 2      # queries per core
NCT = C // 128   # 2 c-tiles
NJ = L // 128    # 16 j-tiles
NI = LQ // 128   # 8 i-chunks
EPS_B = 1e-26

LOG2E = 1.4426950408889634
# Schraudolph bf16: bits = round(logit*128*log2e + 127*128 - 5.6); logit = sim/8
SCH_A = 128.0 * LOG2E / 8.0
SCH_B = 127.0 * 128.0 - 5.6

# exp engine split per (h, jt): 'A' ACT true exp, 'V' DVE Schraudolph.
# (PSUM cannot be DMA'd and Pool cannot read PSUM, so these two engines
# carry all 128 exp tiles; the ratio balances their total load just under
# the PE roofline.) Alternation keeps the 3-deep sim psum ring draining
# on both engines at once.
_ROW8A = ['A', 'V', 'A', 'V', 'A', 'V', 'A', 'V',
          'A', 'V', 'A', 'V', 'A', 'V', 'A', 'V']
_ROW9A = ['A', 'V', 'A', 'V', 'A', 'A', 'V', 'A',
          'V', 'A', 'A', 'V', 'A', 'V', 'A', 'A']
# h0's DVE is busy with the s-folded q/k conversions until ~jt6, so its
# early exps go to ACT.
_ROWH0 = ['A', 'A', 'A', 'A', 'A', 'A', 'A', 'V',
          'A', 'V', 'A', 'V', 'A', 'V', 'A', 'V']
_ROWH7 = ['A', 'V', 'A', 'V', 'A', 'A', 'V', 'A',
          'V', 'A', 'A', 'V', 'A', 'V', 'S', 'S']
EXP_ENG = ([list(_ROWH0)]
           + [list(_ROW9A) if h % 2 else list(_ROW8A)
              for h in range(1, H - 1)]
           + [list(_ROWH7)])


def _body(tc, x, wq, wo, bo, g2, idn, out):
    nc = tc.nc
    from contextlib import ExitStack
    with ExitStack() as ctx:
        ctx.enter_context(nc.allow_low_precision(
            reason="bf16 data path by design"))
        const = ctx.enter_context(tc.tile_pool(name="const", bufs=1))
        bigx = ctx.enter_context(tc.tile_pool(name="bigx", bufs=1))
        sqp = ctx.enter_context(tc.tile_pool(name="sq", bufs=2))
        slnp = ctx.enter_context(tc.tile_pool(name="sln", bufs=2))
        sbnp = ctx.enter_context(tc.tile_pool(name="sbn", bufs=1))
        sbcp = ctx.enter_context(tc.tile_pool(name="sbc", bufs=4))
        qp = ctx.enter_context(tc.tile_pool(name="q", bufs=4))
        kp = ctx.enter_context(tc.tile_pool(name="k", bufs=4))
        vtp = ctx.enter_context(tc.tile_pool(name="vt", bufs=16))
        ptp = ctx.enter_context(tc.tile_pool(name="pt", bufs=2))
        invp = ctx.enter_context(tc.tile_pool(name="inv", bufs=2))
        ytsp = ctx.enter_context(tc.tile_pool(name="yts", bufs=2))
        y4p = ctx.enter_context(tc.tile_pool(name="y4", bufs=4))
        outp = ctx.enter_context(tc.tile_pool(name="outp", bufs=1))
        finp = ctx.enter_context(tc.tile_pool(name="fin", bufs=2))

        # ---------------- constants ----------------
        # Preload the one ACT table that covers every function this kernel
        # uses (Ln, Exp, Copy, Identity). Bacc's auto-inserter is greedy
        # per-function and would thrash 13 loads between the single-function
        # tables; with this explicit load it sees every activation covered.
        from concourse.hw_specs import get_activation_tables
        tabs = list(get_activation_tables(nc.m.arch))
        joint_id = tabs.index("natural_log_exp_and_others")
        nc.scalar.add_instruction(mybir.InstLoadActFuncSet(
            name=nc.get_next_instruction_name(),
            act_func_set_id=joint_id, engine=mybir.EngineType.Activation,
            ins=[], outs=[]))

        dmyr = const.tile([128, 512], BF16, tag="dmyr")
        nc.gpsimd.memset(dmyr[:], 0.0)
        ones128 = const.tile([128, 128], BF16, tag="ones128")
        nc.vector.memset(ones128[:], 1.0)
        ones_row = const.tile([1, 512], BF16, tag="ones_row")
        nc.vector.memset(ones_row[:], 1.0)
        stage = const.tile([128, 8], F32, tag="stage")
        nc.gpsimd.memset(stage[:], 1.0)
        eps_t = const.tile([128, 1], F32, tag="eps")
        nc.gpsimd.memset(eps_t[:], EPS_B)
        zero_t = const.tile([128, 1], F32, tag="zero")
        nc.gpsimd.memset(zero_t[:], 0.0)

        # ---------------- input DMAs ----------------
        # x as four 512-col chunk DMAs, each moving BOTH c-tiles in one
        # transfer (256 dram rows -> [128, 2, 512] sbuf) so only 4 HWDGE
        # setups serialize. wq rides the DVE queue so the ACT sequencer
        # stays free for the norm-chain Ln/Exp.
        x_sb = bigx.tile([128, NCT, L], BF16, tag="x")
        wq_sb = []
        for ct in range(NCT):
            wq_sb.append(const.tile([128, 3 * HID], BF16, tag=f"wq{ct}",
                                    name=f"wq{ct}"))
        nc.sync.dma_start(
            x_sb[:, :, 0:1024],
            x[:, 0:1024].rearrange("(a p) c -> p a c", a=NCT))
        for ct in range(NCT):
            nc.sync.dma_start(wq_sb[ct][:, 0:HID],
                              wq[ct * 128:(ct + 1) * 128, 0:HID])
        nc.sync.dma_start(
            x_sb[:, :, 1024:2048],
            x[:, 1024:2048].rearrange("(a p) c -> p a c", a=NCT))
        for part in range(1, 3):
            for ct in range(NCT):
                nc.sync.dma_start(
                    wq_sb[ct][:, part * HID:(part + 1) * HID],
                    wq[ct * 128:(ct + 1) * 128, part * HID:(part + 1) * HID])

        # idn/wo/bo/g2 are needed only late; ride the gpsimd SWDGE queue.
        idn_sb = const.tile([128, 128], BF16, tag="idn")
        wo_sb = [const.tile([128, C], BF16, tag=f"wo{kt}", name=f"wo{kt}")
                 for kt in range(4)]
        bo_row = const.tile([1, C], BF16, tag="bo_row")
        g2_sb = [const.tile([128, 1], F32, tag=f"g2{mt}", name=f"g2{mt}")
                 for mt in range(2)]

        def _late_dmas():
            nc.gpsimd.dma_start(idn_sb[:], idn)

        def _later_dmas():
            for kt in range(4):
                nc.gpsimd.dma_start(wo_sb[kt][:],
                                    wo[kt * 128:(kt + 1) * 128, :])
            nc.gpsimd.dma_start(bo_row[:], bo)
            for mt in range(2):
                nc.gpsimd.dma_start(g2_sb[mt][:],
                                    g2[mt * 128:(mt + 1) * 128, :])

        q_sb, k_sb = [None] * 4, [None] * 4
        vt_sb = [None] * NJ
        y4_sb = []

        with tc.tile_pool(name="ps", bufs=3, space="PSUM") as psp, \
                tc.tile_pool(name="ps_yt", bufs=2, space="PSUM") as ps_yt:

            def pstile(name):
                return psp.tile([128, 1024], F32, tag="sim", name=name)

            # --- PE pstate warmup: zero matmuls chained on one arena tile
            # while the first x chunks stream in, so the real front matmuls
            # run at full clock. Output is never read. ---
            dmy_ps = pstile("dmy")
            for i in range(7):
                nc.tensor.matmul(dmy_ps[:8, 0:512], ones128[:, 0:8],
                                 dmyr[:], start=True, stop=True)

            # --- chunked input rms-norm: ss (PE bcast-reduce, rows arrive
            # pre-broadcast) -> ln (ACT) -> exp(-.5) (ACT) = rsqrt -> s_bc
            # (row-broadcast, for the per-column q/k scale-folding). Own
            # half (A) in two 512 chunks for a lean chain; far half (B) at
            # 1024 wide. A second, transposed copy s_t [128, NJ] comes from
            # 1-column reduces (out[p, 0] = ss[jt*128+p]) + a tiny ln/exp;
            # it feeds the vT conversions as a per-PARTITION scale, so vT
            # projects from raw x and xn never exists.
            hp = ctx.enter_context(tc.high_priority())
            s_bcA = sbnp.tile([128, 1024], F32, tag="sbcA", name="s_bcA")
            sq8a = sbnp.tile([128, NCT, 1024], BF16, tag="sq8", name="sq8a")
            for c in range(2):
                cs = slice(c * 512, (c + 1) * 512)
                nc.vector.tensor_mul(sq8a[:, :, cs], x_sb[:, :, cs],
                                     x_sb[:, :, cs])
                ssps = ps_yt.tile([128, 512], F32, tag="yt", name=f"ss{c}")
                for ct in range(NCT):
                    nc.tensor.matmul(ssps[:], ones128[:],
                                     sq8a[:, ct, cs],
                                     start=(ct == 0), stop=(ct == NCT - 1))
                s_ln = slnp.tile([128, 512], F32, tag="sln", name=f"sln{c}")
                nc.scalar.activation(s_ln[:], ssps[:], AF.Ln,
                                     bias=eps_t[:], scale=1.0 / C)
                nc.scalar.activation(s_bcA[:, cs], s_ln[:], AF.Exp,
                                     bias=zero_t[:], scale=-0.5)
            sq8b = sbnp.tile([128, NCT, 1024], BF16, tag="sq8b", name="sq8b")
            nc.vector.tensor_mul(sq8b[:], x_sb[:, :, 1024:2048],
                                 x_sb[:, :, 1024:2048])
            # transposed s: one 1-column reduce per key tile. One shared
            # start (pending-zero covers the whole bank region; each
            # column's first write overwrites, the second accumulates).
            sstp = ps_yt.tile([128, NJ], F32, tag="yt", name="sstp")
            for jt in range(NJ):
                sq8v = (sq8a if jt < 8 else sq8b)
                jo = (jt % 8) * 128
                for ct in range(NCT):
                    nc.tensor.matmul(
                        sstp[:, jt:jt + 1],
                        sq8v[:, ct, jo:jo + 128],
                        ones128[:, 0:1],
                        start=(jt == 0 and ct == 0),
                        stop=(jt == NJ - 1 and ct == NCT - 1),
                        skip_group_check=True)
            st_ln = slnp.tile([128, NJ], F32, tag="stln", name="st_ln")
            nc.scalar.activation(st_ln[:], sstp[:], AF.Ln,
                                 bias=eps_t[:], scale=1.0 / C)
            s_t = sbnp.tile([128, NJ], F32, tag="st", name="s_t")
            nc.scalar.activation(s_t[:], st_ln[:], AF.Exp,
                                 bias=zero_t[:], scale=-0.5)
            # key-side norm scale folded into the exp: per-partition scale
            # APs (sim partitions ARE keys), pre-multiplied by the logit
            # scales of the two exp flavors.
            s_t8 = sbnp.tile([128, NJ], F32, tag="st8", name="s_t8")
            nc.vector.tensor_scalar_mul(s_t8[:], s_t[:], 0.125)
            s_tA = sbnp.tile([128, NJ], F32, tag="stA", name="s_tA")
            nc.vector.tensor_scalar_mul(s_tA[:], s_t[:], SCH_A)
            ctx.pop_all().close() if False else None

            # --- projection helpers (psum from the shared arena) ---
            # All projections run on RAW x; the norm scale s is applied at
            # the psum->sbuf conversion (mathematically identical since s
            # is per-column). For q/k that's a per-column mul with the
            # row-broadcast s (DVE tensor_tensor); for vT the column index
            # is the PARTITION, so the transposed s_t rides a per-partition
            # scale and either engine can convert.
            def make_q(mt, split=False):
                ps = pstile(f"qps{mt}")
                for n in range(2):
                    for ct in range(NCT):
                        nc.tensor.matmul(
                            ps[:, n * 512:(n + 1) * 512],
                            wq_sb[ct][:, mt * 128:(mt + 1) * 128],
                            x_sb[:, ct, n * 512:(n + 1) * 512],
                            start=(ct == 0), stop=(ct == NCT - 1))
                t = qp.tile([128, LQ], BF16, tag="q", name=f"qsb{mt}")
                if split:
                    for c in range(2):
                        cs = slice(c * 512, (c + 1) * 512)
                        nc.vector.tensor_mul(t[:, cs], ps[:, cs],
                                             s_bcA[:, cs])
                else:
                    nc.vector.tensor_mul(t[:], ps[:, :], s_bcA[:])
                q_sb[mt] = t

            def make_k_half(mt, half, eng=None):
                if half == 0:
                    k_sb[mt] = kp.tile([128, L], BF16, tag="k",
                                       name=f"ksb{mt}")
                t = k_sb[mt]
                ps = pstile(f"kps{mt}_{half}")
                for n in range(2):
                    for ct in range(NCT):
                        nc.tensor.matmul(
                            ps[:, n * 512:(n + 1) * 512],
                            wq_sb[ct][:, HID + mt * 128:HID + (mt + 1) * 128],
                            x_sb[:, ct,
                                 half * 1024 + n * 512:
                                 half * 1024 + (n + 1) * 512],
                            start=(ct == 0), stop=(ct == NCT - 1))
                if eng is nc.scalar:
                    eng.copy(t[:, half * LQ:(half + 1) * LQ], ps[:, :])
                else:
                    nc.vector.tensor_copy(t[:, half * LQ:(half + 1) * LQ],
                                          ps[:, :])

            def make_vt(jt, eng):
                ps = pstile(f"vps{jt}")
                for ct in range(NCT):
                    nc.tensor.matmul(ps[:, 0:512],
                                     x_sb[:, ct, jt * 128:(jt + 1) * 128],
                                     wq_sb[ct][:, 2 * HID:3 * HID],
                                     start=(ct == 0), stop=(ct == NCT - 1))
                t = vtp.tile([128, H, D + 1], BF16, tag="vt", name=f"vt{jt}")
                src = ps[:, 0:512].rearrange("p (h e) -> p h e", e=D)
                if eng is nc.scalar:
                    eng.activation(t[:, :, 0:D], src, AF.Identity,
                                   bias=zero_t[:], scale=s_t[:, jt:jt + 1])
                else:
                    eng.tensor_scalar_mul(t[:, :, 0:D], src,
                                          s_t[:, jt:jt + 1])
                nc.gpsimd.tensor_copy(
                    t[:, :, D:D + 1].rearrange("p h o -> p (h o)"),
                    stage[:, 0:H])
                vt_sb[jt] = t

            # Minimal prefix for sim h0: q mt0, k mt0 both halves; they
            # depend only on x + wq + the s chain. vT tiles are emitted
            # inside h0's slots.
            make_q(0, split=True)
            make_k_half(0, 0, nc.scalar)
            make_k_half(0, 1, nc.scalar)
            make_vt(0, nc.vector)
            make_vt(1, nc.vector)

            # --- attention: head h sims + exp stream while head h-1's yT
            # accumulates from its fully-materialized pt tile; remaining
            # projections interleave into h0-h3's slots. ---
            pt_sb = [None] * H
            yts_sb = [None] * 4
            out_ps = []
            state = {}

            def sim_chunk(h, jt):
                mt, po = h // 2, (h % 2) * D
                sps = pstile(f"sps{h}_{jt}")
                for n in range(LQ // 512):
                    nc.tensor.matmul(
                        sps[:, n * 512:(n + 1) * 512],
                        k_sb[mt][po:po + D, jt * 128:(jt + 1) * 128],
                        q_sb[mt][po:po + D, n * 512:(n + 1) * 512],
                        start=True, stop=True)
                dst = pt_sb[h][:, jt, :]
                code = EXP_ENG[h][jt]
                stA = s_tA[:, jt:jt + 1]
                st8 = s_t8[:, jt:jt + 1]
                if code == 'V':
                    nc.vector.tensor_scalar(
                        dst.bitcast(U16), sps[:, :], stA, SCH_B,
                        op0=MUL, op1=ADD)
                elif code == 'S':
                    nc.scalar.activation(dst[:, 0:512], sps[:, 0:512],
                                         AF.Exp, bias=zero_t[:], scale=st8)
                    nc.vector.tensor_scalar(
                        dst[:, 512:1024].bitcast(U16), sps[:, 512:1024],
                        stA, SCH_B, op0=MUL, op1=ADD)
                else:
                    nc.scalar.activation(dst, sps[:, :], AF.Exp,
                                         bias=zero_t[:], scale=st8)

            # yT accumulation runs lag-~5-jt behind the SAME head's sim/exp
            # stream (jt-major groups make this legal), so only the last
            # jt-group spills past the head's final sim -- the old
            # one-full-head lag put the whole last yT serially in the tail.
            def yt_alloc(h):
                y0 = ps_yt.tile([128, 4, D + 1], F32, tag="yt",
                                name=f"yt{h}_0")
                y1 = ps_yt.tile([128, 4, D + 1], F32, tag="yt",
                                name=f"yt{h}_1")
                state[h] = {"yps": [y0, y1]}

            def yt_unit(h, g, half):
                """16 matmuls: jt-group 4g..4g+3 x 4 query-chunks into the
                `half` accumulator."""
                yps, pt = state[h]["yps"][half], pt_sb[h]
                for ic in range(4):
                    for jt in range(4 * g, 4 * g + 4):
                        nc.tensor.matmul(
                            yps[:, ic, :],
                            pt[:, jt, (half * 4 + ic) * 128:
                               (half * 4 + ic + 1) * 128],
                            vt_sb[jt][:, h, :],
                            start=(g == 0 and ic == 0 and jt == 0),
                            stop=(jt == NJ - 1),
                            skip_group_check=True)

            def yt_finish(h, half):
                yps = state[h]["yps"][half]
                po = (h % 2) * D
                if half == 0:
                    inv = invp.tile([128, 8], F32, tag="inv",
                                    name=f"inv{h}")
                    state[h]["inv"] = inv
                else:
                    inv = state[h]["inv"]
                nc.vector.reciprocal(
                    inv[:, half * 4:(half + 1) * 4],
                    yps[:, :, D:D + 1].rearrange("p a b -> p (a b)"))
                nc.vector.tensor_mul(
                    yts_sb[h // 2][:, half * 4:(half + 1) * 4, po:po + D],
                    yps[:, :, 0:D],
                    inv[:, half * 4:(half + 1) * 4].unsqueeze(2)
                    .broadcast_to((128, 4, D)))

            def transpose_pair(pair):
                rt = pstile(f"y4ps{pair}")
                y4ps = rt[:].bitcast(BF16)[:, 0:LQ]
                for ich in range(NI):
                    nc.tensor.transpose(y4ps[:, ich * 128:(ich + 1) * 128],
                                        yts_sb[pair][:, ich, :], idn_sb[:])
                y4 = y4p.tile([128, LQ], BF16, tag="y4", name=f"y4_{pair}")
                nc.scalar.copy(y4[:], y4ps)
                y4_sb.append(y4)

            # slot plan: remaining projections and conversions placed into
            # specific (h, jt) slots so their psum use and conv engine time
            # hide under the sim/exp/yt stream. vt emission must stay ahead
            # of the same head's yt units (group g needs vt 4g..4g+3) but
            # behind its Pool xn chunk so PE wait-queue parking is short.
            H0_VT = {1: [2], 2: [3], 4: [4], 5: [5], 8: [6], 9: [7],
                     10: [8], 11: [9], 12: [10], 13: [11], 14: [12]}

            VT_V = frozenset((0, 1, 2, 3, 4, 5, 9, 11, 13, 15))

            def slot(h, jt):
                if h == 0:
                    if jt in H0_VT:
                        for v in H0_VT[jt]:
                            make_vt(v, nc.vector if v in VT_V else nc.scalar)
                elif h == 1:
                    if jt == 0:
                        for v in (13, 14, 15):
                            make_vt(v, nc.vector if v in VT_V else nc.scalar)
                    elif jt == 3:
                        _late_dmas()
                    elif jt == 4:
                        make_q(1)
                    elif jt == 8:
                        make_k_half(1, 0, nc.vector)
                    elif jt == 12:
                        make_k_half(1, 1, nc.scalar)
                elif h == 2:
                    if jt == 6:
                        make_q(2)
                elif h == 3:
                    if jt == 4:
                        make_k_half(2, 0, nc.scalar)
                    elif jt == 10:
                        make_k_half(2, 1, nc.vector)
                elif h == 4:
                    if jt == 6:
                        make_q(3)
                elif h == 5:
                    if jt == 2:
                        _later_dmas()
                    elif jt == 4:
                        make_k_half(3, 0, nc.scalar)
                    elif jt == 10:
                        make_k_half(3, 1, nc.vector)
                elif h == 5 and jt == 2:
                    _later_dmas()

            def out_pair_partial(mt):
                ops = pstile(f"ops{mt}")
                out_ps.append(ops)
                for pair in range(3):
                    for n in range(LQ // 512):
                        nc.tensor.matmul(
                            ops[:, n * 512:(n + 1) * 512],
                            wo_sb[pair][:, mt * 128:(mt + 1) * 128],
                            y4_sb[pair][:, n * 512:(n + 1) * 512],
                            start=(pair == 0), stop=False)
                for n in range(LQ // 512):
                    # rank-1 bias accumulate: + b_out[c] * ones_row
                    nc.tensor.matmul(
                        ops[:, n * 512:(n + 1) * 512],
                        bo_row[0:1, mt * 128:(mt + 1) * 128],
                        ones_row[:], start=False, stop=False,
                        skip_group_check=True)

            # pair-3 transpose: ONE arena tile for both halves (ops0/ops1
            # pin two slots until the final stores, so a second rt tile
            # would serialize behind the single remaining slot).
            t3 = {}

            def transpose3_half(half):
                if half == 0:
                    t3["y4"] = y4p.tile([128, LQ], BF16, tag="y4",
                                        name="y4_3")
                    y4_sb.append(t3["y4"])
                    t3["rt"] = pstile("y4ps3")
                y4ps = t3["rt"][:].bitcast(BF16)[:, 0:LQ]
                hs = slice(half * 512, (half + 1) * 512)
                for i in range(4):
                    ich = half * 4 + i
                    nc.tensor.transpose(y4ps[:, ich * 128:(ich + 1) * 128],
                                        yts_sb[3][:, ich, :], idn_sb[:])
                nc.vector.tensor_copy(t3["y4"][:, hs], y4ps[:, hs])

            YT_UNITS = {7: (0, 0), 8: (0, 1), 11: (1, 0), 12: (1, 1),
                        15: (2, 0)}
            YT_SPILL = {0: (2, 1), 1: (3, 0), 2: (3, 1)}
            for h in range(H + 1):
                if h < H:
                    pt_sb[h] = ptp.tile([128, NJ, LQ], BF16, tag="pt",
                                        name=f"pt{h}")
                    if h % 2 == 0:
                        yts_sb[h // 2] = ytsp.tile([128, NI, 128], BF16,
                                                   tag="yts",
                                                   name=f"yts{h // 2}")
                for jt in range(NJ):
                    if h < H:
                        sim_chunk(h, jt)
                    slot(h, jt)
                    if h < H:
                        if jt == 6:
                            yt_alloc(h)
                        elif jt in YT_UNITS:
                            yt_unit(h, *YT_UNITS[jt])
                    if h > 0 and jt in YT_SPILL:
                        yt_unit(h - 1, *YT_SPILL[jt])
                    if h > 0 and jt in (4, 5) and h <= H:
                        # finishes deferred past the early exps so the DVE
                        # drains the sim ring before the recip+mul pair
                        yt_finish(h - 1, jt - 4)
                    if jt == 8 and h in (2, 4, 6):
                        transpose_pair(h // 2 - 1)
                    if h == H:
                        # out-proj partials first: they depend only on the
                        # long-ready pairs 0-2, so they must not queue
                        # behind the pair-3 transposes in the PE sequencer.
                        if jt == 1:
                            out_pair_partial(0)
                        elif jt == 2:
                            out_pair_partial(1)
                        elif jt == 5:
                            transpose3_half(0)
                        elif jt == 6:
                            transpose3_half(1)

            # ------- pipelined out-proj tail: per 256-col block, finish the
            # pair-3 accumulation, bias, sum-sq, ln/exp rsqrt, scale, store.
            # ss2 accumulators live in the yT psum banks (free by now);
            # block pairs alternate between the two so a block's start=True
            # bank reset never waits on the previous block's Ln read. The
            # bias already rode the out-proj psum, so staging to SBUF is a
            # plain copy; everything downstream reads SBUF (a PSUM-direct
            # tail serializes blocks on bank-level WARs).
            out_sb = outp.tile([128, 2, LQ], BF16, tag="osb")
            ss2ps = [ps_yt.tile([128, 512], F32, tag="yt",
                                name=f"ss2ps{i}") for i in range(2)]
            for nb in range(2):
                bs = slice(nb * 512, (nb + 1) * 512)
                for mt in range(2):
                    nc.tensor.matmul(
                        out_ps[mt][:, bs],
                        wo_sb[3][:, mt * 128:(mt + 1) * 128],
                        t3["y4"][:, bs],
                        start=False, stop=True, skip_group_check=True)
                nc.vector.tensor_copy(out_sb[:, 0, bs], out_ps[0][:, bs])
                nc.scalar.copy(out_sb[:, 1, bs], out_ps[1][:, bs])
                sq2b = sqp.tile([128, 2, 512], BF16, tag="sq2",
                                name=f"sq2_{nb}")
                nc.vector.tensor_mul(sq2b[:], out_sb[:, :, bs],
                                     out_sb[:, :, bs])
                for mt in range(2):
                    nc.tensor.matmul(ss2ps[nb][:], ones128[:],
                                     sq2b[:, mt, :],
                                     start=(mt == 0), stop=(mt == 1))
                s2ln = slnp.tile([128, 512], F32, tag="sln",
                                 name=f"s2ln{nb}")
                nc.scalar.activation(s2ln[:], ss2ps[nb][:], AF.Ln,
                                     bias=eps_t[:], scale=1.0 / C)
                s2bc = sbcp.tile([128, 512], F32, tag="sbc",
                                 name=f"s2bc{nb}")
                nc.scalar.activation(s2bc[:], s2ln[:], AF.Exp,
                                     bias=zero_t[:], scale=-0.5)
                fin = finp.tile([128, 2, 512], F32, tag="fin",
                                name=f"fin{nb}")
                for mt in range(2):
                    nc.vector.scalar_tensor_tensor(
                        fin[:, mt, :], out_sb[:, mt, bs], g2_sb[mt][:],
                        s2bc[:], op0=MUL, op1=MUL)
                for mt in range(2):
                    nc.sync.dma_start(out[mt * 128:(mt + 1) * 128, bs],
                                      fin[:, mt, :])


_NC = None


def _get_nc():
    global _NC
    if _NC is None:
        nc = bacc.Bacc("TRN2", target_bir_lowering=False, debug=False,
                       enable_asserts=False, num_devices=8)
        x_d = nc.dram_tensor("x", [C, L], BF16, kind="ExternalInput")
        wq_d = nc.dram_tensor("wqkvT", [C, 3 * HID], BF16, kind="ExternalInput")
        wo_d = nc.dram_tensor("woutT", [HID, C], BF16, kind="ExternalInput")
        b_d = nc.dram_tensor("bout", [1, C], BF16, kind="ExternalInput")
        g2_d = nc.dram_tensor("g2v", [C, 1], F32, kind="ExternalInput")
        idn_d = nc.dram_tensor("idn", [128, 128], BF16, kind="ExternalInput")
        out_d = nc.dram_tensor("out", [C, LQ], F32, kind="ExternalOutput")
        with tile.TileContext(nc) as tc:
            _body(tc, x_d.ap(), wq_d.ap(), wo_d.ap(), b_d.ap(), g2_d.ap(),
                  idn_d.ap(), out_d.ap())
        nc.compile()
        _NC = nc
    return _NC


def _in_maps(x, g1, w_qkv, w_out, b_out, g2):
    BFH = ml_dtypes.bfloat16
    w2 = (np.asarray(w_qkv, np.float32)
          * np.asarray(g1, np.float32).reshape(1, C))
    wqkvT = np.ascontiguousarray(w2.T).astype(BFH)
    woutT = np.ascontiguousarray(np.asarray(w_out, np.float32).T).astype(BFH)
    bo = np.asarray(b_out, np.float32).reshape(1, C).astype(BFH)
    g2v = np.asarray(g2, np.float32).reshape(C, 1)
    idn = np.eye(128, dtype=BFH)
    maps = []
    for core in range(8):
        b, half = divmod(core, 2)
        xb = np.asarray(x[b], np.float32)
        x_core = np.ascontiguousarray(np.concatenate(
            [xb[:, half * LQ:(half + 1) * LQ],
             xb[:, (1 - half) * LQ:(2 - half) * LQ]], axis=1)).astype(BFH)
        maps.append({"x": x_core, "wqkvT": wqkvT, "woutT": woutT,
                     "bout": bo, "g2v": g2v, "idn": idn})
    return maps


def _assemble(results):
    out = np.empty((B, C, L), np.float32)
    for core in range(8):
        b, half = divmod(core, 2)
        out[b][:, half * LQ:(half + 1) * LQ] = results[core]["out"]
    return out


def kernel(x, g1, w_qkv, w_out, b_out, g2, _trace=False, _tmpdir=None):
    res = run_bass_kernel_spmd(_get_nc(),
                               _in_maps(x, g1, w_qkv, w_out, b_out, g2),
                               core_ids=list(range(8)), trace=_trace,
                               tmpdir=_tmpdir)
    out = _assemble(res.results)
    if _trace:
        return out, res
    return out


# revision 102
# speedup vs baseline: 1.0502x; 1.0029x over previous
"""Trainium2 Bass kernel for the attention module (b=4, c=256, l=2048, h=8, d=64).

Sharding: 8 cores = 4 batches x 2 query-halves (no collectives). Each core
receives its batch's x with columns permuted so its own query half comes
first; it computes k/v for all 2048 key positions and the attention output
for its 1024 queries, then the output projection + final rms-norm.

Device algorithm per core (all matmul data bf16, psum fp32):
  ss     = ones128.T @ bf16(x*x)         (PE bcast-reduce: rows arrive
                                          pre-broadcast; plus 1-col reduces
                                          for a transposed copy s_t)
  s      = exp(-0.5*ln(ss/256+eps))      (ACT Ln+Exp rsqrt: one
                                          natural_log_exp ACT table covers
                                          every function in the kernel, so
                                          exactly one table load, ever)
  q      = bf16((wq.T @ x) * s_bcA)      (norm scale folded at the psum->
                                          sbuf conversion; g1 on host)
  k      = bf16(wk.T @ x)                (UNSCALED: the key-side s rides
                                          the exp as a per-partition scale)
  vT     = bf16((x_slice.T @ wv) * s_t)  (per-partition s_t scale, either
                                          engine; fused ones col for den)
  per head:  simT[j,i] = k_h.T @ q_h     (psum fp32, logits*8*s_j)
             pt = exp(simT*s_t/8)  bf16  (ACT true exp with scale=s_t8 AP /
                                          DVE Schraudolph bitcast-exp with
                                          scalar=s_tA AP, split per table)
             ytps[i, d|den] += pt.T @ vT (jt-major groups, lag ~5 jt behind
                                          the same head's sim/exp stream)
  y_h    = ytps[:, 0:64] * (1/den)       (DVE recip + stride-0 bcast mul)
  y4     = PE transpose(yts, identity)   (bf16, per head-pair)
  out    = woT.T @ y4 (+ b_out as a K=1 rank-1 matmul into the same psum)
  result = out * g2 * exp(-0.5*ln(ss2/256+eps))   (bcast-reduce again)

Scheduling: 512-col-chunked input norm emitted under tc.high_priority so
the list scheduler runs the s chain ahead of the projection matmuls; PE
pstate-warmup dummy matmuls during the initial DMA wait; one rotating
3x[128,1024] psum arena for projections, sim chunks, pair transposes and
the out-proj; yT units interleave into their own head's stream with only
the last jt-group spilling into the next head; per-head exp engine tables
(EXP_ENG) balance ACT/DVE just under the PE roofline, with h7's last two
tiles split across both engines to shorten the tail; q/k projections for
later heads spread across h1-h5 slots; 2x512-col pipelined output tail
(pair-3 out-proj, copy, sum-sq, ln/exp, scale, store). x/wq ride the SP
HWDGE queue (x moves both c-tiles per transfer via an AP rearrange);
idn/wo/bo/g2 ride the gpsimd SWDGE queue late.
"""

import sys

import numpy as np

if "/opt/trn_rl_repo" not in sys.path:
    sys.path.insert(0, "/opt/trn_rl_repo")

import ml_dtypes  # noqa: E402

import concourse.bass as bass  # noqa: E402
import concourse.tile as tile  # noqa: E402
from concourse import bacc, mybir  # noqa: E402
from concourse.bass_utils import run_bass_kernel_spmd  # noqa: E402

F32 = mybir.dt.float32
F32R = mybir.dt.float32r
BF16 = mybir.dt.bfloat16
U16 = mybir.dt.uint16
AF = mybir.ActivationFunctionType
MUL = mybir.AluOpType.mult
ADD = mybir.AluOpType.add

B, C, L = 4, 256, 2048
H, D = 8, 64
HID = H * D
LQ = L // 2      # queries per core
NCT = C // 128   # 2 c-tiles
NJ = L // 128    # 16 j-tiles
NI = LQ // 128   # 8 i-chunks
EPS_B = 1e-26

LOG2E = 1.4426950408889634
# Schraudolph bf16: bits = round(logit*128*log2e + 127*128 - 5.6); logit = sim/8
SCH_A = 128.0 * LOG2E / 8.0
SCH_B = 127.0 * 128.0 - 5.6

# exp engine split per (h, jt): 'A' ACT true exp, 'V' DVE Schraudolph.
# (PSUM cannot be DMA'd and Pool cannot read PSUM, so these two engines
# carry all 128 exp tiles; the ratio balances their total load just under
# the PE roofline.) Alternation keeps the 3-deep sim psum ring draining
# on both engines at once.
_ROW8A = ['A', 'V', 'A', 'V', 'A', 'V', 'A', 'V',
          'A', 'V', 'A', 'V', 'A', 'V', 'A', 'V']
_ROW9A = ['A', 'V', 'A', 'V', 'A', 'A', 'V', 'A',
          'V', 'A', 'A', 'V', 'A', 'V', 'A', 'A']
# h0's DVE is busy with the s-folded q/k conversions until ~jt6, so its
# early exps go to ACT.
_ROWH0 = ['A', 'A', 'A', 'A', 'A', 'A', 'A', 'V',
          'A', 'V', 'A', 'V', 'A', 'V', 'A', 'V']
_ROWH7 = ['A', 'V', 'A', 'V', 'A', 'A', 'V', 'A',
          'V', 'A', 'A', 'V', 'A', 'V', 'S', 'S']
EXP_ENG = ([list(_ROWH0)]
           + [list(_ROW9A) if h % 2 else list(_ROW8A)
              for h in range(1, H - 1)]
           + [list(_ROWH7)])


def _body(tc, x, wq, wo, bo, g2, idn, out):
    nc = tc.nc
    from contextlib import ExitStack
    with ExitStack() as ctx:
        ctx.enter_context(nc.allow_low_precision(
            reason="bf16 data path by design"))
        const = ctx.enter_context(tc.tile_pool(name="const", bufs=1))
        bigx = ctx.enter_context(tc.tile_pool(name="bigx", bufs=1))
        sqp = ctx.enter_context(tc.tile_pool(name="sq", bufs=2))
        slnp = ctx.enter_context(tc.tile_pool(name="sln", bufs=2))
        sbnp = ctx.enter_context(tc.tile_pool(name="sbn", bufs=1))
        sbcp = ctx.enter_context(tc.tile_pool(name="sbc", bufs=4))
        qp = ctx.enter_context(tc.tile_pool(name="q", bufs=4))
        kp = ctx.enter_context(tc.tile_pool(name="k", bufs=4))
        vtp = ctx.enter_context(tc.tile_pool(name="vt", bufs=16))
        ptp = ctx.enter_context(tc.tile_pool(name="pt", bufs=2))
        invp = ctx.enter_context(tc.tile_pool(name="inv", bufs=2))
        ytsp = ctx.enter_context(tc.tile_pool(name="yts", bufs=2))
        y4p = ctx.enter_context(tc.tile_pool(name="y4", bufs=4))
        outp = ctx.enter_context(tc.tile_pool(name="outp", bufs=1))
        finp = ctx.enter_context(tc.tile_pool(name="fin", bufs=2))

        # ---------------- constants ----------------
        # Preload the one ACT table that covers every function this kernel
        # uses (Ln, Exp, Copy, Identity). Bacc's auto-inserter is greedy
        # per-function and would thrash 13 loads between the single-function
        # tables; with this explicit load it sees every activation covered.
        from concourse.hw_specs import get_activation_tables
        tabs = list(get_activation_tables(nc.m.arch))
        joint_id = tabs.index("natural_log_exp_and_others")
        nc.scalar.add_instruction(mybir.InstLoadActFuncSet(
            name=nc.get_next_instruction_name(),
            act_func_set_id=joint_id, engine=mybir.EngineType.Activation,
            ins=[], outs=[]))

        dmyr = const.tile([128, 512], BF16, tag="dmyr")
        nc.gpsimd.memset(dmyr[:], 0.0)
        ones128 = const.tile([128, 128], BF16, tag="ones128")
        nc.vector.memset(ones128[:], 1.0)
        ones_row = const.tile([1, 512], BF16, tag="ones_row")
        nc.vector.memset(ones_row[:], 1.0)
        stage = const.tile([128, 8], F32, tag="stage")
        nc.gpsimd.memset(stage[:], 1.0)
        eps_t = const.tile([128, 1], F32, tag="eps")
        nc.gpsimd.memset(eps_t[:], EPS_B)
        zero_t = const.tile([128, 1], F32, tag="zero")
        nc.gpsimd.memset(zero_t[:], 0.0)

        # ---------------- input DMAs ----------------
        # x as four 512-col chunk DMAs, each moving BOTH c-tiles in one
        # transfer (256 dram rows -> [128, 2, 512] sbuf) so only 4 HWDGE
        # setups serialize. wq rides the DVE queue so the ACT sequencer
        # stays free for the norm-chain Ln/Exp.
        x_sb = bigx.tile([128, NCT, L], BF16, tag="x")
        wq_sb = []
        for ct in range(NCT):
            wq_sb.append(const.tile([128, 3 * HID], BF16, tag=f"wq{ct}",
                                    name=f"wq{ct}"))
        nc.sync.dma_start(
            x_sb[:, :, 0:1024],
            x[:, 0:1024].rearrange("(a p) c -> p a c", a=NCT))
        for ct in range(NCT):
            nc.sync.dma_start(wq_sb[ct][:, 0:HID],
                              wq[ct * 128:(ct + 1) * 128, 0:HID])
        nc.sync.dma_start(
            x_sb[:, :, 1024:2048],
            x[:, 1024:2048].rearrange("(a p) c -> p a c", a=NCT))
        for part in range(1, 3):
            for ct in range(NCT):
                nc.sync.dma_start(
                    wq_sb[ct][:, part * HID:(part + 1) * HID],
                    wq[ct * 128:(ct + 1) * 128, part * HID:(part + 1) * HID])

        # idn/wo/bo/g2 are needed only late; ride the gpsimd SWDGE queue.
        idn_sb = const.tile([128, 128], BF16, tag="idn")
        wo_sb = [const.tile([128, C], BF16, tag=f"wo{kt}", name=f"wo{kt}")
                 for kt in range(4)]
        bo_row = const.tile([1, C], BF16, tag="bo_row")
        g2_sb = [const.tile([128, 1], F32, tag=f"g2{mt}", name=f"g2{mt}")
                 for mt in range(2)]

        def _late_dmas():
            nc.gpsimd.dma_start(idn_sb[:], idn)

        def _later_dmas():
            for kt in range(4):
                nc.gpsimd.dma_start(wo_sb[kt][:],
                                    wo[kt * 128:(kt + 1) * 128, :])
            nc.gpsimd.dma_start(bo_row[:], bo)
            for mt in range(2):
                nc.gpsimd.dma_start(g2_sb[mt][:],
                                    g2[mt * 128:(mt + 1) * 128, :])

        q_sb, k_sb = [None] * 4, [None] * 4
        vt_sb = [None] * NJ
        y4_sb = []

        with tc.tile_pool(name="ps", bufs=3, space="PSUM") as psp, \
                tc.tile_pool(name="ps_yt", bufs=2, space="PSUM") as ps_yt:

            def pstile(name):
                return psp.tile([128, 1024], F32, tag="sim", name=name)

            # --- PE pstate warmup: zero matmuls chained on one arena tile
            # while the first x chunks stream in, so the real front matmuls
            # run at full clock. Output is never read. ---
            dmy_ps = pstile("dmy")
            for i in range(7):
                nc.tensor.matmul(dmy_ps[:8, 0:512], ones128[:, 0:8],
                                 dmyr[:], start=True, stop=True)

            # --- chunked input rms-norm: ss (PE bcast-reduce, rows arrive
            # pre-broadcast) -> ln (ACT) -> exp(-.5) (ACT) = rsqrt -> s_bc
            # (row-broadcast, for the per-column q/k scale-folding). Own
            # half (A) in two 512 chunks for a lean chain; far half (B) at
            # 1024 wide. A second, transposed copy s_t [128, NJ] comes from
            # 1-column reduces (out[p, 0] = ss[jt*128+p]) + a tiny ln/exp;
            # it feeds the vT conversions as a per-PARTITION scale, so vT
            # projects from raw x and xn never exists.
            hp = ctx.enter_context(tc.high_priority())
            s_bcA = sbnp.tile([128, 1024], F32, tag="sbcA", name="s_bcA")
            sq8a = sbnp.tile([128, NCT, 1024], BF16, tag="sq8", name="sq8a")
            for c in range(2):
                cs = slice(c * 512, (c + 1) * 512)
                nc.vector.tensor_mul(sq8a[:, :, cs], x_sb[:, :, cs],
                                     x_sb[:, :, cs])
                ssps = ps_yt.tile([128, 512], F32, tag="yt", name=f"ss{c}")
                for ct in range(NCT):
                    nc.tensor.matmul(ssps[:], ones128[:],
                                     sq8a[:, ct, cs],
                                     start=(ct == 0), stop=(ct == NCT - 1))
                s_ln = slnp.tile([128, 512], F32, tag="sln", name=f"sln{c}")
                nc.scalar.activation(s_ln[:], ssps[:], AF.Ln,
                                     bias=eps_t[:], scale=1.0 / C)
                nc.scalar.activation(s_bcA[:, cs], s_ln[:], AF.Exp,
                                     bias=zero_t[:], scale=-0.5)
            sq8b = sbnp.tile([128, NCT, 1024], BF16, tag="sq8b", name="sq8b")
            nc.vector.tensor_mul(sq8b[:], x_sb[:, :, 1024:2048],
                                 x_sb[:, :, 1024:2048])
            # transposed s: one 1-column reduce per key tile. One shared
            # start (pending-zero covers the whole bank region; each
            # column's first write overwrites, the second accumulates).
            sstp = ps_yt.tile([128, NJ], F32, tag="yt", name="sstp")
            for jt in range(NJ):
                sq8v = (sq8a if jt < 8 else sq8b)
                jo = (jt % 8) * 128
                for ct in range(NCT):
                    nc.tensor.matmul(
                        sstp[:, jt:jt + 1],
                        sq8v[:, ct, jo:jo + 128],
                        ones128[:, 0:1],
                        start=(jt == 0 and ct == 0),
                        stop=(jt == NJ - 1 and ct == NCT - 1),
                        skip_group_check=True)
            st_ln = slnp.tile([128, NJ], F32, tag="stln", name="st_ln")
            nc.scalar.activation(st_ln[:], sstp[:], AF.Ln,
                                 bias=eps_t[:], scale=1.0 / C)
            s_t = sbnp.tile([128, NJ], F32, tag="st", name="s_t")
            nc.scalar.activation(s_t[:], st_ln[:], AF.Exp,
                                 bias=zero_t[:], scale=-0.5)
            # key-side norm scale folded into the exp: per-partition scale
            # APs (sim partitions ARE keys), pre-multiplied by the logit
            # scales of the two exp flavors.
            s_t8 = sbnp.tile([128, NJ], F32, tag="st8", name="s_t8")
            nc.vector.tensor_scalar_mul(s_t8[:], s_t[:], 0.125)
            s_tA = sbnp.tile([128, NJ], F32, tag="stA", name="s_tA")
            nc.vector.tensor_scalar_mul(s_tA[:], s_t[:], SCH_A)
            ctx.pop_all().close() if False else None

            # --- projection helpers (psum from the shared arena) ---
            # All projections run on RAW x; the norm scale s is applied at
            # the psum->sbuf conversion (mathematically identical since s
            # is per-column). For q/k that's a per-column mul with the
            # row-broadcast s (DVE tensor_tensor); for vT the column index
            # is the PARTITION, so the transposed s_t rides a per-partition
            # scale and either engine can convert.
            def make_q(mt, split=False):
                ps = pstile(f"qps{mt}")
                for n in range(2):
                    for ct in range(NCT):
                        nc.tensor.matmul(
                            ps[:, n * 512:(n + 1) * 512],
                            wq_sb[ct][:, mt * 128:(mt + 1) * 128],
                            x_sb[:, ct, n * 512:(n + 1) * 512],
                            start=(ct == 0), stop=(ct == NCT - 1))
                t = qp.tile([128, LQ], BF16, tag="q", name=f"qsb{mt}")
                if split:
                    for c in range(2):
                        cs = slice(c * 512, (c + 1) * 512)
                        nc.vector.tensor_mul(t[:, cs], ps[:, cs],
                                             s_bcA[:, cs])
                else:
                    nc.vector.tensor_mul(t[:], ps[:, :], s_bcA[:])
                q_sb[mt] = t

            def make_k_half(mt, half, eng=None):
                if half == 0:
                    k_sb[mt] = kp.tile([128, L], BF16, tag="k",
                                       name=f"ksb{mt}")
                t = k_sb[mt]
                ps = pstile(f"kps{mt}_{half}")
                for n in range(2):
                    for ct in range(NCT):
                        nc.tensor.matmul(
                            ps[:, n * 512:(n + 1) * 512],
                            wq_sb[ct][:, HID + mt * 128:HID + (mt + 1) * 128],
                            x_sb[:, ct,
                                 half * 1024 + n * 512:
                                 half * 1024 + (n + 1) * 512],
                            start=(ct == 0), stop=(ct == NCT - 1))
                if eng is nc.scalar:
                    eng.copy(t[:, half * LQ:(half + 1) * LQ], ps[:, :])
                else:
                    nc.vector.tensor_copy(t[:, half * LQ:(half + 1) * LQ],
                                          ps[:, :])

            def make_vt(jt, eng):
                ps = pstile(f"vps{jt}")
                for ct in range(NCT):
                    nc.tensor.matmul(ps[:, 0:512],
                                     x_sb[:, ct, jt * 128:(jt + 1) * 128],
                                     wq_sb[ct][:, 2 * HID:3 * HID],
                                     start=(ct == 0), stop=(ct == NCT - 1))
                t = vtp.tile([128, H, D + 1], BF16, tag="vt", name=f"vt{jt}")
                src = ps[:, 0:512].rearrange("p (h e) -> p h e", e=D)
                if eng is nc.scalar:
                    eng.activation(t[:, :, 0:D], src, AF.Identity,
                                   bias=zero_t[:], scale=s_t[:, jt:jt + 1])
                else:
                    eng.tensor_scalar_mul(t[:, :, 0:D], src,
                                          s_t[:, jt:jt + 1])
                nc.gpsimd.tensor_copy(
                    t[:, :, D:D + 1].rearrange("p h o -> p (h o)"),
                    stage[:, 0:H])
                vt_sb[jt] = t

            # Minimal prefix for sim h0: q mt0, k mt0 both halves; they
            # depend only on x + wq + the s chain. vT tiles are emitted
            # inside h0's slots.
            make_q(0, split=True)
            make_k_half(0, 0, nc.scalar)
            make_k_half(0, 1, nc.scalar)
            make_vt(0, nc.vector)
            make_vt(1, nc.vector)

            # --- attention: head h sims + exp stream while head h-1's yT
            # accumulates from its fully-materialized pt tile; remaining
            # projections interleave into h0-h3's slots. ---
            pt_sb = [None] * H
            yts_sb = [None] * 4
            out_ps = []
            state = {}

            def sim_chunk(h, jt):
                mt, po = h // 2, (h % 2) * D
                sps = pstile(f"sps{h}_{jt}")
                for n in range(LQ // 512):
                    nc.tensor.matmul(
                        sps[:, n * 512:(n + 1) * 512],
                        k_sb[mt][po:po + D, jt * 128:(jt + 1) * 128],
                        q_sb[mt][po:po + D, n * 512:(n + 1) * 512],
                        start=True, stop=True)
                dst = pt_sb[h][:, jt, :]
                code = EXP_ENG[h][jt]
                stA = s_tA[:, jt:jt + 1]
                st8 = s_t8[:, jt:jt + 1]
                if code == 'V':
                    nc.vector.tensor_scalar(
                        dst.bitcast(U16), sps[:, :], stA, SCH_B,
                        op0=MUL, op1=ADD)
                elif code == 'S':
                    nc.scalar.activation(dst[:, 0:512], sps[:, 0:512],
                                         AF.Exp, bias=zero_t[:], scale=st8)
                    nc.vector.tensor_scalar(
                        dst[:, 512:1024].bitcast(U16), sps[:, 512:1024],
                        stA, SCH_B, op0=MUL, op1=ADD)
                else:
                    nc.scalar.activation(dst, sps[:, :], AF.Exp,
                                         bias=zero_t[:], scale=st8)

            # yT accumulation runs lag-~5-jt behind the SAME head's sim/exp
            # stream (jt-major groups make this legal), so only the last
            # jt-group spills past the head's final sim -- the old
            # one-full-head lag put the whole last yT serially in the tail.
            def yt_alloc(h):
                y0 = ps_yt.tile([128, 4, D + 1], F32, tag="yt",
                                name=f"yt{h}_0")
                y1 = ps_yt.tile([128, 4, D + 1], F32, tag="yt",
                                name=f"yt{h}_1")
                state[h] = {"yps": [y0, y1]}

            def yt_unit(h, g, half):
                """16 matmuls: jt-group 4g..4g+3 x 4 query-chunks into the
                `half` accumulator."""
                yps, pt = state[h]["yps"][half], pt_sb[h]
                for ic in range(4):
                    for jt in range(4 * g, 4 * g + 4):
                        nc.tensor.matmul(
                            yps[:, ic, :],
                            pt[:, jt, (half * 4 + ic) * 128:
                               (half * 4 + ic + 1) * 128],
                            vt_sb[jt][:, h, :],
                            start=(g == 0 and ic == 0 and jt == 0),
                            stop=(jt == NJ - 1),
                            skip_group_check=True)

            def yt_finish(h, half):
                yps = state[h]["yps"][half]
                po = (h % 2) * D
                if half == 0:
                    inv = invp.tile([128, 8], F32, tag="inv",
                                    name=f"inv{h}")
                    state[h]["inv"] = inv
                else:
                    inv = state[h]["inv"]
                nc.vector.reciprocal(
                    inv[:, half * 4:(half + 1) * 4],
                    yps[:, :, D:D + 1].rearrange("p a b -> p (a b)"))
                nc.vector.tensor_mul(
                    yts_sb[h // 2][:, half * 4:(half + 1) * 4, po:po + D],
                    yps[:, :, 0:D],
                    inv[:, half * 4:(half + 1) * 4].unsqueeze(2)
                    .broadcast_to((128, 4, D)))

            def transpose_pair(pair):
                rt = pstile(f"y4ps{pair}")
                y4ps = rt[:].bitcast(BF16)[:, 0:LQ]
                for ich in range(NI):
                    nc.tensor.transpose(y4ps[:, ich * 128:(ich + 1) * 128],
                                        yts_sb[pair][:, ich, :], idn_sb[:])
                y4 = y4p.tile([128, LQ], BF16, tag="y4", name=f"y4_{pair}")
                nc.scalar.copy(y4[:], y4ps)
                y4_sb.append(y4)

            # slot plan: remaining projections and conversions placed into
            # specific (h, jt) slots so their psum use and conv engine time
            # hide under the sim/exp/yt stream. vt emission must stay ahead
            # of the same head's yt units (group g needs vt 4g..4g+3) but
            # behind its Pool xn chunk so PE wait-queue parking is short.
            H0_VT = {1: [2], 2: [3], 4: [4], 5: [5], 8: [6], 9: [7],
                     10: [8], 11: [9], 12: [10], 13: [11], 14: [12]}

            VT_V = frozenset((0, 1, 2, 3, 4, 5, 9, 11, 13, 15))

            def slot(h, jt):
                if h == 0:
                    if jt in H0_VT:
                        for v in H0_VT[jt]:
                            make_vt(v, nc.vector if v in VT_V else nc.scalar)
                elif h == 1:
                    if jt == 0:
                        for v in (13, 14, 15):
                            make_vt(v, nc.vector if v in VT_V else nc.scalar)
                    elif jt == 3:
                        _late_dmas()
                    elif jt == 4:
                        make_q(1)
                    elif jt == 8:
                        make_k_half(1, 0, nc.vector)
                    elif jt == 12:
                        make_k_half(1, 1, nc.scalar)
                elif h == 2:
                    if jt == 6:
                        make_q(2)
                elif h == 3:
                    if jt == 4:
                        make_k_half(2, 0, nc.scalar)
                    elif jt == 10:
                        make_k_half(2, 1, nc.vector)
                elif h == 4:
                    if jt == 6:
                        make_q(3)
                elif h == 5:
                    if jt == 2:
                        _later_dmas()
                    elif jt == 4:
                        make_k_half(3, 0, nc.scalar)
                    elif jt == 10:
                        make_k_half(3, 1, nc.vector)
                elif h == 5 and jt == 2:
                    _later_dmas()

            def out_pair_partial(mt):
                ops = pstile(f"ops{mt}")
                out_ps.append(ops)
                for pair in range(3):
                    for n in range(LQ // 512):
                        nc.tensor.matmul(
                            ops[:, n * 512:(n + 1) * 512],
                            wo_sb[pair][:, mt * 128:(mt + 1) * 128],
                            y4_sb[pair][:, n * 512:(n + 1) * 512],
                            start=(pair == 0), stop=False)
                for n in range(LQ // 512):
                    # rank-1 bias accumulate: + b_out[c] * ones_row
                    nc.tensor.matmul(
                        ops[:, n * 512:(n + 1) * 512],
                        bo_row[0:1, mt * 128:(mt + 1) * 128],
                        ones_row[:], start=False, stop=False,
                        skip_group_check=True)

            # pair-3 transpose: ONE arena tile for both halves (ops0/ops1
            # pin two slots until the final stores, so a second rt tile
            # would serialize behind the single remaining slot).
            t3 = {}

            def transpose3_half(half):
                if half == 0:
                    t3["y4"] = y4p.tile([128, LQ], BF16, tag="y4",
                                        name="y4_3")
                    y4_sb.append(t3["y4"])
                    t3["rt"] = pstile("y4ps3")
                y4ps = t3["rt"][:].bitcast(BF16)[:, 0:LQ]
                hs = slice(half * 512, (half + 1) * 512)
                for i in range(4):
                    ich = half * 4 + i
                    nc.tensor.transpose(y4ps[:, ich * 128:(ich + 1) * 128],
                                        yts_sb[3][:, ich, :], idn_sb[:])
                nc.vector.tensor_copy(t3["y4"][:, hs], y4ps[:, hs])

            YT_UNITS = {7: (0, 0), 8: (0, 1), 11: (1, 0), 12: (1, 1),
                        15: (2, 0)}
            YT_SPILL = {0: (2, 1), 1: (3, 0), 2: (3, 1)}
            for h in range(H + 1):
                if h < H:
                    pt_sb[h] = ptp.tile([128, NJ, LQ], BF16, tag="pt",
                                        name=f"pt{h}")
                    if h % 2 == 0:
                        yts_sb[h // 2] = ytsp.tile([128, NI, 128], BF16,
                                                   tag="yts",
                                                   name=f"yts{h // 2}")
                for jt in range(NJ):
                    if h < H:
                        sim_chunk(h, jt)
                    slot(h, jt)
                    if h < H:
                        if jt == 6:
                            yt_alloc(h)
                        elif jt in YT_UNITS:
                            yt_unit(h, *YT_UNITS[jt])
                    if h > 0 and jt in YT_SPILL:
                        yt_unit(h - 1, *YT_SPILL[jt])
                    if h > 0 and jt in (4, 5) and h <= H:
                        # finishes deferred past the early exps so the DVE
                        # drains the sim ring before the recip+mul pair
                        yt_finish(h - 1, jt - 4)
                    if jt == 8 and h in (2, 4, 6):
                        transpose_pair(h // 2 - 1)
                    if h == H:
                        # out-proj partials first: they depend only on the
                        # long-ready pairs 0-2, so they must not queue
                        # behind the pair-3 transposes in the PE sequencer.
                        if jt == 1:
                            out_pair_partial(0)
                        elif jt == 2:
                            out_pair_partial(1)
                        elif jt == 5:
                            transpose3_half(0)
                        elif jt == 6:
                            transpose3_half(1)

            # ------- pipelined out-proj tail: per 256-col block, finish the
            # pair-3 accumulation, bias, sum-sq, ln/exp rsqrt, scale, store.
            # ss2 accumulators live in the yT psum banks (free by now);
            # block pairs alternate between the two so a block's start=True
            # bank reset never waits on the previous block's Ln read. The
            # bias already rode the out-proj psum, so staging to SBUF is a
            # plain copy; everything downstream reads SBUF (a PSUM-direct
            # tail serializes blocks on bank-level WARs).
            out_sb = outp.tile([128, 2, LQ], BF16, tag="osb")
            ss2ps = [ps_yt.tile([128, 512], F32, tag="yt",
                                name=f"ss2ps{i}") for i in range(2)]
            for nb in range(2):
                bs = slice(nb * 512, (nb + 1) * 512)
                for mt in range(2):
                    nc.tensor.matmul(
                        out_ps[mt][:, bs],
                        wo_sb[3][:, mt * 128:(mt + 1) * 128],
                        t3["y4"][:, bs],
                        start=False, stop=True, skip_group_check=True)
                nc.vector.tensor_copy(out_sb[:, 0, bs], out_ps[0][:, bs])
                nc.scalar.copy(out_sb[:, 1, bs], out_ps[1][:, bs])
                sq2b = sqp.tile([128, 2, 512], BF16, tag="sq2",
                                name=f"sq2_{nb}")
                nc.vector.tensor_mul(sq2b[:], out_sb[:, :, bs],
                                     out_sb[:, :, bs])
                for mt in range(2):
                    nc.tensor.matmul(ss2ps[nb][:], ones128[:],
                                     sq2b[:, mt, :],
                                     start=(mt == 0), stop=(mt == 1))
                s2ln = slnp.tile([128, 512], F32, tag="sln",
                                 name=f"s2ln{nb}")
                nc.scalar.activation(s2ln[:], ss2ps[nb][:], AF.Ln,
                                     bias=eps_t[:], scale=1.0 / C)
                s2bc = sbcp.tile([128, 512], F32, tag="sbc",
                                 name=f"s2bc{nb}")
                nc.scalar.activation(s2bc[:], s2ln[:], AF.Exp,
                                     bias=zero_t[:], scale=-0.5)
                fin = finp.tile([128, 2, 512], BF16, tag="fin",
                                name=f"fin{nb}")
                for mt in range(2):
                    nc.vector.scalar_tensor_tensor(
                        fin[:, mt, :], out_sb[:, mt, bs], g2_sb[mt][:],
                        s2bc[:], op0=MUL, op1=MUL)
                for mt in range(2):
                    nc.sync.dma_start(out[mt * 128:(mt + 1) * 128, bs],
                                      fin[:, mt, :])


_NC = None


def _get_nc():
    global _NC
    if _NC is None:
        nc = bacc.Bacc("TRN2", target_bir_lowering=False, debug=False,
                       enable_asserts=False, num_devices=8)
        x_d = nc.dram_tensor("x", [C, L], BF16, kind="ExternalInput")
        wq_d = nc.dram_tensor("wqkvT", [C, 3 * HID], BF16, kind="ExternalInput")
        wo_d = nc.dram_tensor("woutT", [HID, C], BF16, kind="ExternalInput")
        b_d = nc.dram_tensor("bout", [1, C], BF16, kind="ExternalInput")
        g2_d = nc.dram_tensor("g2v", [C, 1], F32, kind="ExternalInput")
        idn_d = nc.dram_tensor("idn", [128, 128], BF16, kind="ExternalInput")
        out_d = nc.dram_tensor("out", [C, LQ], BF16, kind="ExternalOutput")
        with tile.TileContext(nc) as tc:
            _body(tc, x_d.ap(), wq_d.ap(), wo_d.ap(), b_d.ap(), g2_d.ap(),
                  idn_d.ap(), out_d.ap())
        nc.compile()
        _NC = nc
    return _NC


def _in_maps(x, g1, w_qkv, w_out, b_out, g2):
    BFH = ml_dtypes.bfloat16
    w2 = (np.asarray(w_qkv, np.float32)
          * np.asarray(g1, np.float32).reshape(1, C))
    wqkvT = np.ascontiguousarray(w2.T).astype(BFH)
    woutT = np.ascontiguousarray(np.asarray(w_out, np.float32).T).astype(BFH)
    bo = np.asarray(b_out, np.float32).reshape(1, C).astype(BFH)
    g2v = np.asarray(g2, np.float32).reshape(C, 1)
    idn = np.eye(128, dtype=BFH)
    maps = []
    for core in range(8):
        b, half = divmod(core, 2)
        xb = np.asarray(x[b], np.float32)
        x_core = np.ascontiguousarray(np.concatenate(
            [xb[:, half * LQ:(half + 1) * LQ],
             xb[:, (1 - half) * LQ:(2 - half) * LQ]], axis=1)).astype(BFH)
        maps.append({"x": x_core, "wqkvT": wqkvT, "woutT": woutT,
                     "bout": bo, "g2v": g2v, "idn": idn})
    return maps


def _assemble(results):
    out = np.empty((B, C, L), np.float32)
    for core in range(8):
        b, half = divmod(core, 2)
        out[b][:, half * LQ:(half + 1) * LQ] = np.asarray(
            results[core]["out"], dtype=np.float32)
    return out


def kernel(x, g1, w_qkv, w_out, b_out, g2, _trace=False, _tmpdir=None):
    res = run_bass_kernel_spmd(_get_nc(),
                               _in_maps(x, g1, w_qkv, w_out, b_out, g2),
                               core_ids=list(range(8)), trace=_trace,
                               tmpdir=_tmpdir)
    out = _assemble(res.results)
    if _trace:
        return out, res
    return out
